# revision 123
# baseline (speedup 1.0000x reference)
"""Multi-head attention (B=1, S=2048, H=1024, NH=16) on 8 trn2 NeuronCores.

Sharding: head-parallel. Core c owns heads {2c, 2c+1} (= 128 of the 1024
hidden dims). Each core computes its Q/K/V projection slices, the full
attention for its 2 heads, and a full-width partial of the output
projection (contraction over its 128 context dims). Host sums the 8
partials and adds the (host-folded) biases.

Attention elementwise path (the reference quirk: masked scores are set
to 0 pre-softmax, so masked lanes contribute exp(0)=1):

    E = m*exp(s/8) + (1-m)            (m in {0,1})
      = m*(e0 - 1) + 1,   e0 = exp(s/8)

  * Act engine: e0 = Exp(s_psum / 8) straight out of PSUM -> SBUF bf16,
    one 1024-col tile per iteration; Act is the body's rate limiter
    (64 x 1038 ns) and does ~nothing else until the epilogue.
  * Masked-combine, one of three per-tile paths (Pool has no PSUM
    access; fused stt runs 1x on DVE; plain tensor_scalar/tensor_tensor
    hit DVE 4x/2x modes with all-bf16 SBUF operands):
      - fp8 stt (1127ns DVE): fused (e0-1)*m, fp8 mask - most evens +
        panel-0 odd j<=9.
      - tsp+Pool: t = e0 - 1 on DVE at 4x (327ns), eh = t*m on Pool
        (2.2us, deep pipeline slack) - tiles placed in Pool's idle
        windows: ph0-even j0/2/4/8 (k=0..24) and all ph1 odds. ph0
        j12/j14 are dual-loaded (fp8 for h0, bf16 copy for h1) so h1's
        tiles ride Pool where DVE otherwise drifts behind Act.
      - tsp+DVE-mult (921ns): tiles feeding each head-handoff's norm
        chain (k=27-31) and the last tiles (61,63), where Pool's queue
        latency would stall the o-buffer handoff or the tail.
    Load-balance invariant: DVE+Act carry exp (66.4us) + combines +
    evictions ~ 146us between them; every avoidable DVE ns matters
    because Act can run only e_p=8 exps ahead (e0-buffer WAR), so mask
    lateness or DVE drift surfaces directly as Act stalls.
  * The "+1" term: sum_j 1*vaug[j,:] = colsum(Vaug) = C, an i-independent
    vector, injected into each PV PSUM accumulation as a single K=2
    matmul against host-precomputed C split into bf16 hi+lo rows.

Loop structure: one flat 64-iteration pipeline over (panel, head,
j-block) with panels of 1024 queries. S-matmuls are emitted one
iteration ahead (priority-0). PV pipeline depth: 14 while V streams
(k<30), dipping to 8 at k=36-41 to pull the p0h1->p1h0 norm handoff
ahead of its consumers, 9 mid, tapering to 3 at the end. V-projection
chunks ride k=13..28. Panel-0's y-chunks ride odd iterations k>=43
(oT panel 0 lands ~k=41 via xbar-transposes), their DMAs in row-pair
groups; 5 leftover chunks go just before the PV flush. Panel-1's y is
the tail: norm -> PE+identity transposes -> 16 chunks on DVE/Act with
column-half DMAs, last rows as 1-row DMAs. PSUM: s 2x2 + o 2 + y/v
2x1 = 8 banks.

DMA discipline (~360 GB/s serial transfer device, one HWDGE slot per
dma_start): the queue is deadline-sorted (EDF). Hard deadlines: x
loads gate proj->S->exp directly - prologue streams wk/wq + xk0/xq0/
xq1 as 256-token half-quarters chased by half-panel projections
(first exp ~13.5us), then xk1..3, then v quarters (PE-blocking via
the v-proj window). Soft deadlines: stt masks are due t(k + e_p) via
the e0-WAR; Pool-path masks only gate PV (t(k + depth)) and load in
the post-xq3 stream. Masks live in packed 1024-col slot tiles
(fp8 mask_e / bf16 mask_o + mo8/me16), y leaves in 0.25-0.5MB groups
as chunks complete. q/k biases fold into projection evictions.

Precision: identical to the reference-faithful baseline - all matmuls
bf16 with fp32 PSUM accumulation, softmax without max-subtraction
(exponent ~ N(0,0.33^2) cannot overflow). Modeled 110.5us (was 112.6);
hw rel err 1.4e-3.
"""

import math

import numpy as np
import ml_dtypes

BF16 = ml_dtypes.bfloat16
FP8 = ml_dtypes.float8_e4m3
S, H, NH, DK = 2048, 1024, 16, 64
NCORES = 8
HPC = NH // NCORES          # heads per core = 2
DPC = HPC * DK              # head dims per core = 128
KC = H // 128               # contraction chunks = 8
TP = 2                      # 1024-wide query token panels
JC = S // 128               # 128-wide key chunks = 16
IC = 1024 // 128            # i-chunks per panel = 8
VA = DK + 1                 # v columns + ones column = 65

_CACHE = {}

# schedule knobs (tuned via TimelineSim sweeps)
EARLY_DEPTH = 14     # PV pipeline depth while v-quarters stream
MID_DEPTH = 9        # depth after the early taper
END_DEPTH = 3        # post-loop drain backlog
P0_START = 43        # first in-body panel-0 y-chunk iteration (odd)
DVEMUL = (27, 29, 31, 61, 63)  # odd-j tiles multiplied on DVE
HANDOFF_PAUSE = 2    # iterations to pause draining after a head's last j
DIP_DEPTH = 8        # temporary depth dip at k=36-41 (handoff pull-in)
NORM_PRIO = 15     # priority offset for head-norm DVE ops (o-WAR release)


def _oslc(ic):
    """o_ps column offset for ic-th 65-wide slice: 7 slices in bank 0,
    the 8th at 512 so no matmul crosses a PSUM bank boundary."""
    b, r = divmod(ic, 7)
    return b * 512 + r * VA


def _build_program():
    """Build + compile the (identical) per-core Bass program."""
    from contextlib import ExitStack

    import concourse.bacc as bacc
    import concourse.tile as tile
    from concourse import mybir

    dt = mybir.dt
    AF = mybir.ActivationFunctionType
    ALU = mybir.AluOpType
    f8 = dt.float8e4

    nc = bacc.Bacc("TRN2", target_bir_lowering=False, debug=False)

    qT_d = nc.dram_tensor("qT", [H, S], dt.bfloat16, kind="ExternalInput").ap()
    kT_d = nc.dram_tensor("kT", [H, S], dt.bfloat16, kind="ExternalInput").ap()
    vT_d = nc.dram_tensor("vT", [H, S], dt.bfloat16, kind="ExternalInput").ap()
    maskT8_d = nc.dram_tensor("maskT8", [S, S], f8, kind="ExternalInput").ap()
    maskTb_d = nc.dram_tensor("maskTb", [S, S], dt.bfloat16, kind="ExternalInput").ap()
    wk_d = nc.dram_tensor("wk", [128, KC * DPC], dt.bfloat16, kind="ExternalInput").ap()
    wq_d = nc.dram_tensor("wq", [128, KC * DPC], dt.bfloat16, kind="ExternalInput").ap()
    wv_d = nc.dram_tensor("wv", [128, KC * DPC], dt.bfloat16, kind="ExternalInput").ap()
    wo_d = nc.dram_tensor("wo", [DPC, H], dt.bfloat16, kind="ExternalInput").ap()
    bqk_d = nc.dram_tensor("bqk", [128, 2], dt.float32, kind="ExternalInput").ap()
    cv_d = nc.dram_tensor("cvec", [2, HPC * VA], dt.bfloat16, kind="ExternalInput").ap()
    id_d = nc.dram_tensor("ident", [128, 128], dt.bfloat16, kind="ExternalInput").ap()
    yT_d = nc.dram_tensor("yT", [H, S], dt.bfloat16, kind="ExternalOutput").ap()

    with tile.TileContext(nc) as tc, ExitStack() as ctx:
        cp = ctx.enter_context(tc.tile_pool(name="const", bufs=1))
        e_p = ctx.enter_context(tc.tile_pool(name="ex", bufs=8))
        eh_p = ctx.enter_context(tc.tile_pool(name="ehat", bufs=15))
        rc_p = ctx.enter_context(tc.tile_pool(name="recip", bufs=2))
        t_p = ctx.enter_context(tc.tile_pool(name="tmul", bufs=3))
        mh_p = ctx.enter_context(tc.tile_pool(name="maskhi", bufs=1))
        otp_p = ctx.enter_context(tc.tile_pool(name="otpan", bufs=2))
        vin_p = ctx.enter_context(tc.tile_pool(name="vin", bufs=1))
        xy_p = ctx.enter_context(tc.tile_pool(name="xy", bufs=3))

        # ---- DMA schedule: wk | xk quarters | wq | xq quarters | rest ----
        wk_sb = cp.tile([128, KC * DPC], dt.bfloat16, tag="wk")
        nc.sync.dma_start(out=wk_sb, in_=wk_d)
        # preload the Exp activation table off the critical path
        warm = cp.tile([1, 2], dt.bfloat16, tag="warm")
        nc.vector.memset(warm, 0.0)
        nc.scalar.activation(warm, warm, AF.Exp)

        # ---- token-streamed inputs: x loads in token quarters so each
        # kT/qT panel completes as its quarter lands; S(j) needs only
        # kT token-block j and qT's active panel half, so attention
        # starts ~15us earlier. Late panels project as body side-work.
        wq_sb = cp.tile([128, KC * DPC], dt.bfloat16, tag="wq")
        bqk_sb = cp.tile([128, 2], dt.float32, tag="bqk")
        cv_sb = cp.tile([2, HPC * VA], dt.bfloat16, tag="cv")
        ident = cp.tile([128, 128], dt.bfloat16, tag="ident")
        qT_sb = cp.tile([128, S], dt.bfloat16, tag="qTs")
        kT_sb = cp.tile([128, S], dt.bfloat16, tag="kTs")
        vaug = cp.tile([128, JC * (HPC * VA)], dt.bfloat16, tag="vaug")
        nc.gpsimd.memset(
            vaug.rearrange("p (a v) -> p a v", v=VA)[:, :, DK:VA], 1.0
        )
        ot_pan = None
        oT_full = cp.tile([128, S], dt.bfloat16, tag="oTfull")
        y_pan = {}
        # packed mask slot layouts (1024-col slots), only the (j, ph)
        # combinations actually consumed from each dtype
        E_SLOT = {(6, 0): 0, (10, 0): 1, (12, 0): 2,
                  (14, 0): 3, (0, 1): 4, (2, 1): 5, (4, 1): 6, (6, 1): 7,
                  (8, 1): 8, (10, 1): 9, (12, 1): 10, (14, 1): 11}
        O_SLOT = {(11, 0): 0, (13, 0): 1, (15, 0): 2, (1, 1): 3, (3, 1): 4,
                  (5, 1): 5, (7, 1): 6, (9, 1): 7, (11, 1): 8, (13, 1): 9,
                  (15, 1): 10, (12, 0): 11, (14, 0): 12}
        mask_e = cp.tile([128, 12 * 1024], f8, tag="maske")
        mask_o = mh_p.tile([128, 13 * 1024], dt.bfloat16, tag="masko")
        wv_sb = cp.tile([128, KC * DPC], dt.bfloat16, tag="wv")
        wo_sb = cp.tile([128, H], dt.bfloat16, tag="wo")

        xtq = {}

        def x_tq(pre, x_d, c):
            xt = xy_p.tile(
                [128, KC * 1024], dt.bfloat16, tag="xy", name=f"x{pre}{c}"
            )[:, : KC * 512]
            nc.sync.dma_start(
                out=xt.rearrange("p (a i) -> p a i", a=KC),
                in_=x_d[:, c * 512 : (c + 1) * 512].rearrange(
                    "(a p) i -> p a i", p=128
                ),
            )
            xtq[pre, c] = xt

        def x_tq_half(pre, x_d, c, hf):
            """256-token half-quarter load (finer prologue pipelining)."""
            if (pre, c) not in xtq:
                xtq[pre, c] = xy_p.tile(
                    [128, KC * 1024], dt.bfloat16, tag="xy", name=f"x{pre}{c}"
                )[:, : KC * 512]
            nc.sync.dma_start(
                out=xtq[pre, c].rearrange("p (a i) -> p a i", a=KC)[
                    :, :, hf * 256 : (hf + 1) * 256
                ],
                in_=x_d[
                    :, c * 512 + hf * 256 : c * 512 + (hf + 1) * 256
                ].rearrange("(a p) i -> p a i", p=128),
            )

        def mask_cols(j0, nb, ph):
            """Load nb j-blocks of parity j0%2 starting at j0, cols half ph."""
            par = j0 % 2
            t, d, smap = (
                (mask_e, maskT8_d, E_SLOT) if par == 0
                else (mask_o, maskTb_d, O_SLOT)
            )
            slot = smap[j0, ph]
            nc.sync.dma_start(
                out=t.rearrange("p (b i) -> p b i", i=1024)[
                    :, slot : slot + nb, :
                ],
                in_=d.rearrange("(b two p) i -> p two b i", two=2, p=128)[
                    :, par, j0 // 2 : j0 // 2 + nb, ph * 1024 : (ph + 1) * 1024
                ],
            )

        # early odd-j (panel-0) mask slices staged fp8: they ride the fused
        # stt path, saving deadline-critical early DMA
        mo8 = cp.tile([128, 5 * 1024], f8, tag="mo8")
        # ph0 evens j0, j2, j4, j8 staged bf16: their tiles (k=0,2,4,8 and
        # 16,18,20,24) ride the tsp+Pool path in Pool's idle windows,
        # relieving DVE's 1127ns stt load where it drifts behind Act
        ME_SLOT = {0: 0, 2: 1, 4: 2, 8: 3}
        me16 = cp.tile([128, 4 * 1024], dt.bfloat16, tag="me16")

        def mask_slc(j, ph, h=0):
            if ph == 0 and j % 2 == 1 and j <= 9:
                return mo8[:, (j // 2) * 1024 : (j // 2 + 1) * 1024]
            if ph == 0 and j in ME_SLOT:
                s_ = ME_SLOT[j]
                return me16[:, s_ * 1024 : (s_ + 1) * 1024]
            if ph == 0 and h == 1 and j in (12, 14):
                # h1's copy of these slices is bf16 (Pool path); h0 uses fp8
                slot = O_SLOT[j, 0]
                return mask_o[:, slot * 1024 : (slot + 1) * 1024]
            t, smap = (mask_e, E_SLOT) if j % 2 == 0 else (mask_o, O_SLOT)
            slot = smap[j, ph]
            return t[:, slot * 1024 : (slot + 1) * 1024]

        vin = []

        def v_quarter(c):
            t_ = vin_p.tile(
                [128, KC * 512], dt.bfloat16, tag=f"vq{c % 3}", name=f"vq{c}"
            )
            nc.sync.dma_start(
                out=t_.rearrange("p (a i) -> p a i", a=KC),
                in_=vT_d[:, c * 512 : (c + 1) * 512].rearrange(
                    "(a p) i -> p a i", p=128
                ),
            )
            vin.append(t_)

        # DMA queue order: deadline-sorted (EDF) just-in-time stream over
        # the serial ~360GB/s transfer device. x-loads gate the S->exp
        # chain directly (hard deadlines); masks/v have pipeline slack.
        x_tq_half("k", kT_d, 0, 0)
        nc.sync.dma_start(out=wq_sb, in_=wq_d)
        nc.sync.dma_start(out=bqk_sb, in_=bqk_d)
        x_tq_half("q", qT_d, 0, 0)
        x_tq_half("q", qT_d, 0, 1)
        x_tq_half("q", qT_d, 1, 0)
        x_tq_half("q", qT_d, 1, 1)
        x_tq_half("k", kT_d, 0, 1)
        x_tq("k", kT_d, 1)
        x_tq_half("k", kT_d, 2, 0)
        x_tq_half("k", kT_d, 2, 1)
        nc.sync.dma_start(              # j0, j2 ph0 bf16 (Pool path)
            out=me16[:, 0 : 2 * 1024].rearrange("p (b i) -> p b i", i=1024),
            in_=maskTb_d.rearrange("(b two p) i -> p two b i", two=2, p=128)[
                :, 0, 0:2, 0:1024
            ],
        )
        nc.sync.dma_start(              # j1,3,5,7,9 ph0 as fp8
            out=mo8.rearrange("p (b i) -> p b i", i=1024),
            in_=maskT8_d.rearrange("(b two p) i -> p two b i", two=2, p=128)[
                :, 1, 0:5, 0:1024
            ],
        )
        x_tq_half("k", kT_d, 3, 0)
        x_tq_half("k", kT_d, 3, 1)
        nc.sync.dma_start(out=wv_sb, in_=wv_d)
        nc.sync.dma_start(              # j4 ph0 bf16 (Pool path)
            out=me16[:, 2 * 1024 : 3 * 1024],
            in_=maskTb_d.rearrange("(b two p) i -> p two b i", two=2, p=128)[
                :, 0, 2, 0:1024
            ],
        )
        mask_cols(6, 1, 0)              # j6 (fp8)
        v_quarter(0)
        nc.sync.dma_start(              # j8 ph0 bf16 (Pool path)
            out=me16[:, 3 * 1024 : 4 * 1024],
            in_=maskTb_d.rearrange("(b two p) i -> p two b i", two=2, p=128)[
                :, 0, 4, 0:1024
            ],
        )
        mask_cols(10, 1, 0)             # j10 (fp8)
        v_quarter(1)
        mask_cols(11, 2, 0)             # j11, j13 (bf16)
        mask_cols(12, 2, 0)             # j12, j14 (fp8)
        v_quarter(2)
        mask_cols(15, 1, 0)             # j15 (bf16)
        v_quarter(3)
        x_tq_half("q", qT_d, 2, 0)
        x_tq_half("q", qT_d, 2, 1)
        x_tq_half("q", qT_d, 3, 0)
        x_tq_half("q", qT_d, 3, 1)
        nc.sync.dma_start(              # j12, j14 ph0 bf16 copies (h1 Pool)
            out=mask_o.rearrange("p (b i) -> p b i", i=1024)[:, 11:13, :],
            in_=maskTb_d.rearrange("(b two p) i -> p two b i", two=2, p=128)[
                :, 0, 6:8, 0:1024
            ],
        )
        nc.sync.dma_start(out=cv_sb, in_=cv_d)
        mask_cols(0, 2, 1)              # ph1 evens j0, j2
        mask_cols(1, 2, 1)              # ph1 odds j1, j3
        mask_cols(4, 2, 1)
        mask_cols(5, 2, 1)
        nc.sync.dma_start(out=wo_sb, in_=wo_d)
        mask_cols(8, 2, 1)
        mask_cols(9, 2, 1)
        mask_cols(12, 2, 1)
        mask_cols(13, 2, 1)
        nc.sync.dma_start(out=ident, in_=id_d)

        ones2 = cp.tile([2, 128], dt.bfloat16, tag="ones2")
        nc.vector.memset(ones2, 1.0)

        # ---- attention: PSUM = s 2x2 + o 1x2 + y/v 2x1 = 8 banks ----
        with tc.tile_pool(name="ps_s", bufs=2, space="PSUM") as ps_p, \
             tc.tile_pool(name="ps_o", bufs=1, space="PSUM") as po_p, \
             tc.tile_pool(name="ps_v", bufs=2, space="PSUM") as pv_p:

            def v_proj_chunk(t):
                """Token-chunk t of the V projection into vaug."""
                ps = pv_p.tile([128, DPC], dt.float32, tag="vps", name=f"pv{t}")
                c, ts_ = divmod(t, 4)
                for kk in range(KC):
                    nc.tensor.matmul(
                        ps,
                        lhsT=vin[c][:, kk * 512 + ts_ * 128 : kk * 512 + (ts_ + 1) * 128],
                        rhs=wv_sb[:, kk * DPC : (kk + 1) * DPC],
                        start=(kk == 0),
                        stop=(kk == KC - 1),
                    )
                base = t * (HPC * VA)
                dst = vaug[:, base : base + HPC * VA].rearrange(
                    "p (a v) -> p a v", v=VA
                )[:, :, 0:DK]
                src = ps.rearrange("p (a d) -> p a d", d=DK)
                if t >= 12:
                    # late chunks evict on Act: DVE is the pacer by then
                    # and Act idles waiting on the panel-1 q stream
                    nc.scalar.activation(dst, src, AF.Copy)
                else:
                    nc.vector.tensor_copy(dst, src)

            def pv_mms(h, j, et, o_ps):
                for ic in range(IC):
                    nc.tensor.matmul(
                        o_ps[:, _oslc(ic) : _oslc(ic) + VA],
                        lhsT=et[:, ic * 128 : (ic + 1) * 128],
                        rhs=vaug[:, j * (HPC * VA) + h * VA : j * (HPC * VA) + (h + 1) * VA],
                        start=(j == 0 and ic % 7 == 0),
                        stop=False,
                    )

            def c_inject(h, o_ps):
                """+C (hi+lo rows); last slice per bank carries the stop."""
                for ic in range(IC):
                    nc.tensor.matmul(
                        o_ps[:, _oslc(ic) : _oslc(ic) + VA],
                        lhsT=ones2,
                        rhs=cv_sb[:, h * VA : (h + 1) * VA],
                        start=False,
                        stop=(ic in (6, 7)),
                    )

            import concourse.bass as bass_mod

            def norm_bank(p, h, o_ps, ot_pan, b, prio=False):
                """Normalize one PSUM bank of o_ps into ot_pan. With prio,
                schedule the DVE ops early in the stream: the norm releases
                the o-PSUM buffer the next head's PV (and thus the whole PE
                stream) waits on."""
                ctx2 = tc.high_priority(offset=NORM_PRIO) if prio else None
                if ctx2 is not None:
                    ctx2.__enter__()
                try:
                    n_ic = (7, 1)[b]
                    rc = rc_p.tile(
                        [128, 8], dt.float32, tag="rc", name=f"rc{p}_{h}_{b}"
                    )
                    den = bass_mod.AP(
                        tensor=o_ps.tensor,
                        offset=o_ps.offset + b * 512 + DK,
                        ap=[o_ps.ap[0], [VA, n_ic]],
                    )
                    nc.vector.reciprocal(rc[:, :n_ic], den)
                    src_ap = bass_mod.AP(
                        tensor=o_ps.tensor,
                        offset=o_ps.offset + b * 512,
                        ap=[o_ps.ap[0], [VA, n_ic], [1, DK]],
                    )
                    rcb = bass_mod.AP(
                        tensor=rc.tensor,
                        offset=rc.offset,
                        ap=[rc.ap[0], [1, n_ic], [0, DK]],
                    )
                    dst = bass_mod.AP(
                        tensor=ot_pan.tensor,
                        offset=ot_pan.offset + b * 7 * 128 + h * DK,
                        ap=[ot_pan.ap[0], [128, n_ic], [1, DK]],
                    )
                    nc.vector.tensor_mul(dst, src_ap, rcb)
                finally:
                    if ctx2 is not None:
                        ctx2.__exit__(None, None, None)

            def o_chunk(p, nn, half, eng, pool=None, tag="vps"):
                """One 512-col y chunk: matmul + eviction into y_pan."""
                y_ps = (pool or pv_p).tile(
                    [128, 512], dt.float32, tag=tag, name=f"y{p}_{nn}_{half}"
                )
                nc.tensor.matmul(
                    y_ps,
                    lhsT=wo_sb[:, nn * 128 : (nn + 1) * 128],
                    rhs=oT_full[:, p * 1024 + half * 512 : p * 1024 + (half + 1) * 512],
                    start=True,
                    stop=True,
                )
                eng.tensor_copy(
                    y_pan[p][:, nn * 1024 + half * 512 : nn * 1024 + (half + 1) * 512],
                    y_ps,
                )

            def y_dma(p, lo, hi):
                """DMA y_pan[p] rows nn in [lo,hi) out to yT."""
                nc.sync.dma_start(
                    out=yT_d[lo * 128 : hi * 128, p * 1024 : (p + 1) * 1024]
                    .rearrange("(a p2) i -> p2 a i", p2=128),
                    in_=y_pan[p][:, lo * 1024 : hi * 1024]
                    .rearrange("p (a i) -> p a i", i=1024),
                )

            # ---- flat 64-iteration pipeline ----
            ot_map = {}

            def emit_s(k):
                p, h, j = k // 32, (k // 16) % 2, k % 16
                hs = h * DK
                s_ps = ps_p.tile(
                    [128, 1024], dt.float32, tag="sps", name=f"s{k}"
                )
                with tc.high_priority():
                    for q in range(2):
                        nc.tensor.matmul(
                            s_ps[:, q * 512 : (q + 1) * 512],
                            lhsT=kT_sb[hs : hs + DK, j * 128 : (j + 1) * 128],
                            rhs=qT_sb[hs : hs + DK,
                                      p * 1024 + q * 512 : p * 1024 + (q + 1) * 512],
                            start=True,
                            stop=True,
                        )
                return s_ps

            def transposes(p):
                # batched xbar transposes, bank-0's 7 chunks first
                nc.sync.dma_start_transpose(
                    out=oT_full[:, p * 1024 : p * 1024 + 896].rearrange(
                        "p2 (b c) -> p2 b c", c=128
                    ),
                    in_=ot_map[p][:, 0:896],
                )
                nc.sync.dma_start_transpose(
                    out=oT_full[:, p * 1024 + 896 : (p + 1) * 1024],
                    in_=ot_map[p][:, 896:1024],
                )

            def norm_banks(p, h, o_ps, ot_pan):
                norm_bank(p, h, o_ps, ot_pan, 0)
                norm_bank(p, h, o_ps, ot_pan, 1)

            o_ps_map = {}
            pend = []
            dstate = {"pause": 0}

            def drain_one():
                pp, ph, pj, peh = pend.pop(0)
                if (pp, ph) not in o_ps_map:
                    o_ps_map[pp, ph] = po_p.tile(
                        [128, 1024], dt.float32, tag="ops", name=f"ops{pp}{ph}"
                    )
                o_ps = o_ps_map[pp, ph]
                pv_mms(ph, pj, peh, o_ps)
                if pj == JC - 1:
                    c_inject(ph, o_ps)
                    if (pp, ph) != (TP - 1, HPC - 1):
                        norm_banks(pp, ph, o_ps, ot_map[pp])
                        if ph == HPC - 1:
                            transposes(pp)
                        dstate["pause"] = HANDOFF_PAUSE

            def proj_panel(pre, c):
                w, dest = (wk_sb, kT_sb) if pre == "k" else (wq_sb, qT_sb)
                bcol = bqk_sb[:, 1:2] if pre == "k" else bqk_sb[:, 0:1]
                ps = pv_p.tile(
                    [128, 512], dt.float32, tag="vps", name=f"pp{pre}{c}"
                )
                for kk in range(KC):
                    nc.tensor.matmul(
                        ps,
                        lhsT=w[:, kk * DPC : (kk + 1) * DPC],
                        rhs=xtq[pre, c][:, kk * 512 : (kk + 1) * 512],
                        start=(kk == 0),
                        stop=(kk == KC - 1),
                    )
                # DVE eviction: an Act Identity here would displace an exp
                nc.vector.tensor_scalar_add(
                    dest[:, c * 512 : (c + 1) * 512], ps, bcol
                )

            def proj_panel_half(pre, c, hf, act=False):
                """256-token half-panel projection (prologue pipelining)."""
                w, dest = (wk_sb, kT_sb) if pre == "k" else (wq_sb, qT_sb)
                bcol = bqk_sb[:, 1:2] if pre == "k" else bqk_sb[:, 0:1]
                ps = pv_p.tile(
                    [128, 256], dt.float32, tag="vps", name=f"ph{pre}{c}{hf}"
                )
                lo = hf * 256
                for kk in range(KC):
                    nc.tensor.matmul(
                        ps,
                        lhsT=w[:, kk * DPC : (kk + 1) * DPC],
                        rhs=xtq[pre, c][:, kk * 512 + lo : kk * 512 + lo + 256],
                        start=(kk == 0),
                        stop=(kk == KC - 1),
                    )
                if act:
                    nc.scalar.activation(
                        dest[:, c * 512 + lo : c * 512 + lo + 256], ps,
                        AF.Identity, bias=bcol,
                    )
                else:
                    nc.vector.tensor_scalar_add(
                        dest[:, c * 512 + lo : c * 512 + lo + 256], ps, bcol
                    )

            proj_panel_half("k", 0, 0)
            proj_panel_half("q", 0, 0)
            proj_panel_half("q", 0, 1)
            # S(0)'s first half only needs qT cols 0-511: run it while the
            # xq1 halves are still streaming in
            s0 = ps_p.tile([128, 1024], dt.float32, tag="sps", name="s0")
            with tc.high_priority():
                nc.tensor.matmul(
                    s0[:, 0:512],
                    lhsT=kT_sb[0:DK, 0:128],
                    rhs=qT_sb[0:DK, 0:512],
                    start=True,
                    stop=True,
                )
            proj_panel_half("q", 1, 0)
            proj_panel_half("q", 1, 1)

            def p0_chunk(ck, eng):
                """One 512-col panel-0 y chunk; eviction on `eng`."""
                nn, half = ck // 2, ck % 2
                y_ps = pv_p.tile(
                    [128, 512], dt.float32, tag="vps", name=f"y0_{ck}"
                )
                nc.tensor.matmul(
                    y_ps,
                    lhsT=wo_sb[:, nn * 128 : (nn + 1) * 128],
                    rhs=oT_full[:, half * 512 : (half + 1) * 512],
                    start=True,
                    stop=True,
                )
                if eng is nc.scalar:
                    nc.scalar.activation(
                        y_pan[0][:, nn * 1024 + half * 512 :
                                 nn * 1024 + (half + 1) * 512],
                        y_ps, AF.Copy,
                    )
                else:
                    eng.tensor_copy(
                        y_pan[0][:, nn * 1024 + half * 512 :
                                 nn * 1024 + (half + 1) * 512],
                        y_ps,
                    )

            # per-iteration elementwise path:
            #  - fp8 stt (fused, 1x DVE): all even-j + panel-0 odd j<=9
            #  - tsp(e0-1) 4x + DVE 2x mult: tiles feeding the panel-0 norm
            #    chain (pull eh27-31 early) and the last tiles (Pool lags)
            #  - tsp(e0-1) 4x + Pool mult: everything else
            # DVE-mult tiles: the last odd-j of each head feed the norm ->
            # next-head-PV chain (o-PSUM buffer reuse); Pool's lag there
            # would stall the S stream at every head handoff
            DVEMUL_K = set(DVEMUL)

            def fp8_k(k):
                j = k % 16
                if k < 32 and j in (0, 2, 4, 8):
                    return False  # bf16, tsp+Pool in Pool's idle windows
                if 16 <= k < 32 and j in (12, 14):
                    return False  # dual-loaded: h1 copy is bf16 Pool path
                return j % 2 == 0 or (k < 32 and j <= 9)

            def depth(k):
                """PV pipeline depth: deep early (v-load slack), shallower
                mid (pulls the panel-0 norm chain ahead of its y consumers),
                tapering at the end so the post-loop backlog stays short."""
                if k < 30:
                    return EARLY_DEPTH
                if k < 52:
                    d = max(MID_DEPTH, EARLY_DEPTH - 2 * (k - 29))
                    if 36 <= k < 42:
                        d = min(d, DIP_DEPTH)
                    return d
                return max(END_DEPTH, MID_DEPTH - (k - 51))

            # S(0)'s second half, then the k0b half-projection
            with tc.high_priority():
                nc.tensor.matmul(
                    s0[:, 512:1024],
                    lhsT=kT_sb[0:DK, 0:128],
                    rhs=qT_sb[0:DK, 512:1024],
                    start=True,
                    stop=True,
                )
            s_next = s0
            proj_panel_half("k", 0, 1)
            for k in range(64):
                p, h, j = k // 32, (k // 16) % 2, k % 16
                if p not in ot_map:
                    ot_map[p] = otp_p.tile(
                        [128, IC * 128], dt.bfloat16, tag="otp", name=f"otp{p}"
                    )
                    y_pan[p] = xy_p.tile(
                        [128, KC * 1024], dt.bfloat16, tag="xy", name=f"ypan{p}"
                    )
                s_ps = s_next
                e0 = e_p.tile(
                    [128, 1024], dt.bfloat16, tag="e0", name=f"e0_{k}"
                )
                nc.scalar.activation(e0, s_ps, AF.Exp, scale=1.0 / math.sqrt(DK))
                eh = eh_p.tile(
                    [128, 1024], dt.bfloat16, tag="eh", name=f"eh{k}"
                )
                mslc = mask_slc(j, p, h)
                if fp8_k(k):
                    nc.vector.scalar_tensor_tensor(
                        eh, e0, 1.0, mslc, ALU.subtract, ALU.mult
                    )
                else:
                    t = t_p.tile(
                        [128, 1024], dt.bfloat16, tag="tm", name=f"tm{k}"
                    )
                    nc.vector.tensor_scalar_add(t, e0, -1.0)
                    if k in DVEMUL_K:
                        nc.vector.tensor_mul(eh, t, mslc)
                    else:
                        nc.gpsimd.tensor_mul(eh, t, mslc)
                # next S ahead of PV/side work so Act is never starved
                if k + 1 < 64:
                    s_next = emit_s(k + 1)
                # side work riding this iteration
                if k == 2:
                    proj_panel("k", 1)
                elif k == 6:
                    proj_panel_half("k", 2, 0)
                    proj_panel_half("k", 2, 1)
                elif k == 10:
                    proj_panel_half("k", 3, 0)
                    proj_panel_half("k", 3, 1)
                elif k == 28:
                    proj_panel_half("q", 2, 0, act=True)
                    proj_panel_half("q", 2, 1, act=True)
                elif k == 30:
                    proj_panel_half("q", 3, 0, act=True)
                    proj_panel_half("q", 3, 1, act=True)
                if 13 <= k <= 28:
                    v_proj_chunk(k - 13)
                # panel-0 y chunks ride the odd iterations of the second
                # half (oT_full panel 0 lands ~k=41); their DMAs go out in
                # row-pair groups as soon as both halves of a pair exist
                if P0_START <= k and k % 2 == 1:
                    ck = (k - P0_START) // 2
                    p0_chunk(ck, nc.vector)
                    if ck % 4 == 3:
                        y_dma(0, ck // 2 - 1, ck // 2 + 1)
                # variable-depth software pipeline for PV; after a head's
                # last j-block drains, pause 2 iterations so the norm ->
                # o-buffer-reuse chain overlaps the S stream instead of
                # stalling the next head's first PV
                if dstate["pause"] > 0:
                    dstate["pause"] -= 1
                else:
                    while len(pend) >= depth(k):
                        drain_one()
                        if dstate["pause"]:
                            break
                pend.append((p, h, j, eh))

            # remaining panel-0 y chunks: emitted before the PV flush so
            # their matmuls keep PE hot while the last PV/norm chain runs
            n_inb = max(0, (63 - P0_START) // 2 + 1)
            rows_dmad = 2 * sum(1 for c2 in range(n_inb) if c2 % 4 == 3)
            for ck in range(n_inb, 16):
                p0_chunk(ck, nc.vector if ck % 2 == 0 else nc.scalar)
                if ck % 2 == 1 and (ck + 1) // 2 - rows_dmad >= 2:
                    y_dma(0, rows_dmad, (ck + 1) // 2)
                    rows_dmad = (ck + 1) // 2
            if rows_dmad < 8:
                y_dma(0, rows_dmad, 8)

            while pend:
                drain_one()

            # ---- tail: panel-1 epilogue with PE transposes (PE and
            # all engines idle here; skips the 3us DMA-xbar latency) ----
            def y_dma_cols(p, half, lo, hi):
                nc.sync.dma_start(
                    out=yT_d[lo * 128 : hi * 128,
                             p * 1024 + half * 512 : p * 1024 + (half + 1) * 512]
                    .rearrange("(a p2) i -> p2 a i", p2=128),
                    in_=y_pan[p].rearrange("p (a i) -> p a i", i=1024)[
                        :, lo:hi, half * 512 : (half + 1) * 512
                    ],
                )

            o_ps = o_ps_map[TP - 1, HPC - 1]
            ot1 = ot_map[TP - 1]
            rr = (nc.vector, nc.scalar)

            def pe_transpose(lic):
                tp = ps_p.tile(
                    [128, 128], dt.bfloat16, tag="sps", name=f"tp{lic}"
                )
                nc.tensor.transpose(tp, ot1[:, lic * 128 : (lic + 1) * 128], ident)
                eng = nc.vector
                dst = oT_full[:, 1024 + lic * 128 : 1024 + (lic + 1) * 128]
                if eng is nc.scalar:
                    nc.scalar.activation(dst, tp, AF.Copy)
                else:
                    eng.tensor_copy(dst, tp)

            def tail_chunk(ck, half, nn=None):
                nn = ck % 8 if nn is None else nn
                eng = rr[ck % 2]
                pool, tag = (pv_p, "vps") if ck % 2 == 0 else (ps_p, "sps")
                if eng is nc.scalar:
                    y_ps = pool.tile(
                        [128, 512], dt.float32, tag=tag, name=f"y1_{ck}"
                    )
                    nc.tensor.matmul(
                        y_ps,
                        lhsT=wo_sb[:, nn * 128 : (nn + 1) * 128],
                        rhs=oT_full[:, 1024 + half * 512 : 1024 + (half + 1) * 512],
                        start=True,
                        stop=True,
                    )
                    nc.scalar.activation(
                        y_pan[1][:, nn * 1024 + half * 512 :
                                 nn * 1024 + (half + 1) * 512],
                        y_ps, AF.Copy,
                    )
                else:
                    o_chunk(1, nn, half, eng, pool=pool, tag=tag)

            norm_bank(TP - 1, HPC - 1, o_ps, ot1, 0, prio=True)
            for lic in range(7):
                pe_transpose(lic)
            for ck in range(8):
                tail_chunk(ck, 0)
                if ck == 3:
                    y_dma_cols(1, 0, 0, 4)
            y_dma_cols(1, 0, 4, 8)
            norm_bank(TP - 1, HPC - 1, o_ps, ot1, 1, prio=True)
            pe_transpose(7)
            for ck in range(8, 16):
                tail_chunk(ck, 1)
                if ck == 11:
                    y_dma_cols(1, 1, 0, 4)
                elif ck == 13:
                    y_dma_cols(1, 1, 4, 6)
                elif ck == 14:
                    y_dma_cols(1, 1, 6, 7)
            y_dma_cols(1, 1, 7, 8)

    nc.compile()
    return nc


def get_program():
    if "nc" not in _CACHE:
        _CACHE["nc"] = _build_program()
    return _CACHE["nc"]


def _wshuf(wT):
    """[1024 k, 128 n] -> [128 p, KC*128] with chunk kk at cols kk*128."""
    return np.ascontiguousarray(
        wT.reshape(KC, 128, DPC).transpose(1, 0, 2).reshape(128, KC * DPC)
    ).astype(BF16)


def make_in_maps(query, key, value, attention_mask, Wq, bq, Wk, bk, Wv, Wo):
    """Host-side sharding: per-core input dicts."""
    qT = np.ascontiguousarray(np.asarray(query, np.float32)[0].T).astype(BF16)
    kT = np.ascontiguousarray(np.asarray(key, np.float32)[0].T).astype(BF16)
    vT = np.ascontiguousarray(np.asarray(value, np.float32)[0].T).astype(BF16)
    maskTf = np.ascontiguousarray(np.asarray(attention_mask, np.float32)[0, 0].T)
    maskT8 = maskTf.astype(FP8)
    maskTb = maskTf.astype(BF16)
    # C = colsum(Vaug) per head = [colsum(value) @ Wv_h.T | S], fp64 on host,
    # split into bf16 hi+lo rows for near-fp32 injection accuracy
    vcol = np.asarray(value, np.float64)[0].sum(axis=0)  # [H]

    in_maps = []
    for c in range(NCORES):
        ns = slice(c * DPC, (c + 1) * DPC)
        cfull = vcol @ np.asarray(Wv, np.float64)[ns].T  # [DPC]
        cvec = np.zeros((2, HPC * VA), np.float64)
        for h in range(HPC):
            cvec[0, h * VA : h * VA + DK] = cfull[h * DK : (h + 1) * DK]
            cvec[0, h * VA + DK] = float(S)
        chi = cvec.astype(BF16)
        clo = (cvec - chi.astype(np.float64)).astype(BF16)
        cboth = np.concatenate([chi[0:1], clo[0:1]], axis=0)
        bqk = np.stack(
            [np.asarray(bq, np.float32)[ns], np.asarray(bk, np.float32)[ns]],
            axis=1,
        )
        in_maps.append(
            {
                "qT": qT,
                "kT": kT,
                "vT": vT,
                "maskT8": maskT8,
                "maskTb": maskTb,
                "wq": _wshuf(np.asarray(Wq, np.float32)[ns].T),
                "wk": _wshuf(np.asarray(Wk, np.float32)[ns].T),
                "wv": _wshuf(np.asarray(Wv, np.float32)[ns].T),
                "wo": np.ascontiguousarray(np.asarray(Wo, np.float32)[:, ns].T).astype(BF16),
                "bqk": np.ascontiguousarray(bqk),
                "cvec": cboth,
                "ident": np.eye(128, dtype=BF16),
            }
        )
    return in_maps


def combine_outputs(results, Wv_bias, Wo, bo):
    """Sum per-core partial yT's (bf16 -> fp32), add host-folded biases."""
    acc = np.zeros((H, S), np.float32)
    for r in results:
        acc += r["yT"].astype(np.float32)
    bias = np.asarray(bo, np.float32) + np.asarray(Wv_bias, np.float32) @ np.asarray(
        Wo, np.float32
    ).T
    return (acc.T + bias[None, :]).astype(np.float32)[None]


def kernel(
    query,
    key,
    value,
    attention_mask,
    Wq,
    bq,
    Wk,
    bk,
    Wv,
    bv,
    Wo,
    bo,
    head,
    hidden_size,
):
    from concourse.bass_utils import run_bass_kernel_spmd

    nc = get_program()
    in_maps = make_in_maps(
        query, key, value, attention_mask, Wq, bq, Wk, bk, Wv, Wo
    )
    res = run_bass_kernel_spmd(nc, in_maps, list(range(NCORES)))
    return combine_outputs(res.results, bv, Wo, bo)



# revision 124
# speedup vs baseline: 1.0032x; 1.0032x over previous
"""Multi-head attention (B=1, S=2048, H=1024, NH=16) on 8 trn2 NeuronCores.

Sharding: head-parallel. Core c owns heads {2c, 2c+1} (= 128 of the 1024
hidden dims). Each core computes its Q/K/V projection slices, the full
attention for its 2 heads, and a full-width partial of the output
projection (contraction over its 128 context dims). Host sums the 8
partials and adds the (host-folded) biases.

Attention elementwise path (the reference quirk: masked scores are set
to 0 pre-softmax, so masked lanes contribute exp(0)=1):

    E = m*exp(s/8) + (1-m)            (m in {0,1})
      = m*(e0 - 1) + 1,   e0 = exp(s/8)

  * Act engine: e0 = Exp(s_psum / 8) straight out of PSUM -> SBUF bf16,
    one 1024-col tile per iteration; Act is the body's rate limiter
    (64 x 1038 ns) and does ~nothing else until the epilogue.
  * Masked-combine, one of three per-tile paths (Pool has no PSUM
    access; fused stt runs 1x on DVE; plain tensor_scalar/tensor_tensor
    hit DVE 4x/2x modes with all-bf16 SBUF operands):
      - fp8 stt (1127ns DVE): fused (e0-1)*m, fp8 mask - most evens +
        panel-0 odd j<=9.
      - tsp+Pool: t = e0 - 1 on DVE at 4x (327ns), eh = t*m on Pool
        (2.2us, deep pipeline slack) - tiles placed in Pool's idle
        windows: ph0-even j0/2/4/8 (k=0..24) and all ph1 odds. ph0
        j12/j14 are dual-loaded (fp8 for h0, bf16 copy for h1) so h1's
        tiles ride Pool where DVE otherwise drifts behind Act.
      - tsp+DVE-mult (921ns): tiles feeding each head-handoff's norm
        chain (k=27-31) and the last tiles (61,63), where Pool's queue
        latency would stall the o-buffer handoff or the tail.
    Load-balance invariant: DVE+Act carry exp (66.4us) + combines +
    evictions ~ 146us between them; every avoidable DVE ns matters
    because Act can run only e_p=8 exps ahead (e0-buffer WAR), so mask
    lateness or DVE drift surfaces directly as Act stalls.
  * The "+1" term: sum_j 1*vaug[j,:] = colsum(Vaug) = C, an i-independent
    vector, injected into each PV PSUM accumulation as a single K=2
    matmul against host-precomputed C split into bf16 hi+lo rows.

Loop structure: one flat 64-iteration pipeline over (panel, head,
j-block) with panels of 1024 queries. S-matmuls are emitted one
iteration ahead (priority-0). PV pipeline depth: 14 while V streams
(k<30), dipping to 8 at k=36-41 to pull the p0h1->p1h0 norm handoff
ahead of its consumers, 9 mid, tapering to 3 at the end. V-projection
chunks ride k=13..28. Panel-0's y-chunks ride odd iterations k>=43
(oT panel 0 lands ~k=41 via xbar-transposes), their DMAs in row-pair
groups; 5 leftover chunks go just before the PV flush. Panel-1's y is
the tail: norm -> PE+identity transposes -> 16 chunks on DVE/Act with
column-half DMAs, last rows as 1-row DMAs. PSUM: s 2x2 + o 2 + y/v
2x1 = 8 banks.

DMA discipline (~360 GB/s serial transfer device, one HWDGE slot per
dma_start): the queue is deadline-sorted (EDF). Hard deadlines: x
loads gate proj->S->exp directly - prologue streams wk/wq + xk0/xq0/
xq1 as 256-token half-quarters chased by half-panel projections
(first exp ~13.5us), then xk1..3, then v quarters (PE-blocking via
the v-proj window). Soft deadlines: stt masks are due t(k + e_p) via
the e0-WAR; Pool-path masks only gate PV (t(k + depth)) and load in
the post-xq3 stream. Masks live in packed 1024-col slot tiles
(fp8 mask_e / bf16 mask_o + mo8/me16), y leaves in 0.25-0.5MB groups
as chunks complete. q/k biases fold into projection evictions.

Precision: identical to the reference-faithful baseline - all matmuls
bf16 with fp32 PSUM accumulation, softmax without max-subtraction
(exponent ~ N(0,0.33^2) cannot overflow). Modeled 110.5us (was 112.6);
hw rel err 1.4e-3.
"""

import math

import numpy as np
import ml_dtypes

BF16 = ml_dtypes.bfloat16
FP8 = ml_dtypes.float8_e4m3
S, H, NH, DK = 2048, 1024, 16, 64
NCORES = 8
HPC = NH // NCORES          # heads per core = 2
DPC = HPC * DK              # head dims per core = 128
KC = H // 128               # contraction chunks = 8
TP = 2                      # 1024-wide query token panels
JC = S // 128               # 128-wide key chunks = 16
IC = 1024 // 128            # i-chunks per panel = 8
VA = DK + 1                 # v columns + ones column = 65

_CACHE = {}

# schedule knobs (tuned via TimelineSim sweeps)
EARLY_DEPTH = 14     # PV pipeline depth while v-quarters stream
MID_DEPTH = 9        # depth after the early taper
END_DEPTH = 3        # post-loop drain backlog
P0_START = 43        # first in-body panel-0 y-chunk iteration (odd)
DVEMUL = (27, 29, 31, 61, 63)  # odd-j tiles multiplied on DVE
HANDOFF_PAUSE = 2    # iterations to pause draining after a head's last j
DIP_DEPTH = 8        # temporary depth dip at k=36-41 (handoff pull-in)
NORM_PRIO = 15     # priority offset for head-norm DVE ops (o-WAR release)


def _oslc(ic):
    """o_ps column offset for ic-th 65-wide slice: 7 slices in bank 0,
    the 8th at 512 so no matmul crosses a PSUM bank boundary."""
    b, r = divmod(ic, 7)
    return b * 512 + r * VA


def _build_program():
    """Build + compile the (identical) per-core Bass program."""
    from contextlib import ExitStack

    import concourse.bacc as bacc
    import concourse.tile as tile
    from concourse import mybir

    dt = mybir.dt
    AF = mybir.ActivationFunctionType
    ALU = mybir.AluOpType
    f8 = dt.float8e4

    nc = bacc.Bacc("TRN2", target_bir_lowering=False, debug=False)

    qT_d = nc.dram_tensor("qT", [H, S], dt.bfloat16, kind="ExternalInput").ap()
    kT_d = nc.dram_tensor("kT", [H, S], dt.bfloat16, kind="ExternalInput").ap()
    vT_d = nc.dram_tensor("vT", [H, S], dt.bfloat16, kind="ExternalInput").ap()
    maskT8_d = nc.dram_tensor("maskT8", [S, S], f8, kind="ExternalInput").ap()
    maskTb_d = nc.dram_tensor("maskTb", [S, S], dt.bfloat16, kind="ExternalInput").ap()
    wk_d = nc.dram_tensor("wk", [128, KC * DPC], dt.bfloat16, kind="ExternalInput").ap()
    wq_d = nc.dram_tensor("wq", [128, KC * DPC], dt.bfloat16, kind="ExternalInput").ap()
    wv_d = nc.dram_tensor("wv", [128, KC * DPC], dt.bfloat16, kind="ExternalInput").ap()
    wo_d = nc.dram_tensor("wo", [DPC, H], dt.bfloat16, kind="ExternalInput").ap()
    bqk_d = nc.dram_tensor("bqk", [128, 2], dt.float32, kind="ExternalInput").ap()
    cv_d = nc.dram_tensor("cvec", [2, HPC * VA], dt.bfloat16, kind="ExternalInput").ap()
    id_d = nc.dram_tensor("ident", [128, 128], dt.bfloat16, kind="ExternalInput").ap()
    yT_d = nc.dram_tensor("yT", [H, S], dt.bfloat16, kind="ExternalOutput").ap()

    with tile.TileContext(nc) as tc, ExitStack() as ctx:
        cp = ctx.enter_context(tc.tile_pool(name="const", bufs=1))
        e_p = ctx.enter_context(tc.tile_pool(name="ex", bufs=8))
        eh_p = ctx.enter_context(tc.tile_pool(name="ehat", bufs=15))
        rc_p = ctx.enter_context(tc.tile_pool(name="recip", bufs=2))
        t_p = ctx.enter_context(tc.tile_pool(name="tmul", bufs=3))
        mh_p = ctx.enter_context(tc.tile_pool(name="maskhi", bufs=1))
        otp_p = ctx.enter_context(tc.tile_pool(name="otpan", bufs=2))
        vin_p = ctx.enter_context(tc.tile_pool(name="vin", bufs=1))
        xy_p = ctx.enter_context(tc.tile_pool(name="xy", bufs=3))

        # ---- DMA schedule: wk | xk quarters | wq | xq quarters | rest ----
        wk_sb = cp.tile([128, KC * DPC], dt.bfloat16, tag="wk")
        nc.sync.dma_start(out=wk_sb, in_=wk_d)
        # preload the Exp activation table off the critical path
        warm = cp.tile([1, 2], dt.bfloat16, tag="warm")
        nc.vector.memset(warm, 0.0)
        nc.scalar.activation(warm, warm, AF.Exp)

        # ---- token-streamed inputs: x loads in token quarters so each
        # kT/qT panel completes as its quarter lands; S(j) needs only
        # kT token-block j and qT's active panel half, so attention
        # starts ~15us earlier. Late panels project as body side-work.
        wq_sb = cp.tile([128, KC * DPC], dt.bfloat16, tag="wq")
        bqk_sb = cp.tile([128, 2], dt.float32, tag="bqk")
        cv_sb = cp.tile([2, HPC * VA], dt.bfloat16, tag="cv")
        ident = cp.tile([128, 128], dt.bfloat16, tag="ident")
        qT_sb = cp.tile([128, S], dt.bfloat16, tag="qTs")
        kT_sb = cp.tile([128, S], dt.bfloat16, tag="kTs")
        vaug = cp.tile([128, JC * (HPC * VA)], dt.bfloat16, tag="vaug")
        nc.gpsimd.memset(
            vaug.rearrange("p (a v) -> p a v", v=VA)[:, :, DK:VA], 1.0
        )
        ot_pan = None
        oT_full = cp.tile([128, S], dt.bfloat16, tag="oTfull")
        y_pan = {}
        # packed mask slot layouts (1024-col slots), only the (j, ph)
        # combinations actually consumed from each dtype
        E_SLOT = {(6, 0): 0, (10, 0): 1, (12, 0): 2,
                  (14, 0): 3, (0, 1): 4, (2, 1): 5, (4, 1): 6, (6, 1): 7,
                  (8, 1): 8, (10, 1): 9, (12, 1): 10, (14, 1): 11}
        O_SLOT = {(11, 0): 0, (13, 0): 1, (15, 0): 2, (1, 1): 3, (3, 1): 4,
                  (5, 1): 5, (7, 1): 6, (9, 1): 7, (11, 1): 8, (13, 1): 9,
                  (15, 1): 10, (12, 0): 11, (14, 0): 12}
        mask_e = cp.tile([128, 12 * 1024], f8, tag="maske")
        mask_o = mh_p.tile([128, 13 * 1024], dt.bfloat16, tag="masko")
        wv_sb = cp.tile([128, KC * DPC], dt.bfloat16, tag="wv")
        wo_sb = cp.tile([128, H], dt.bfloat16, tag="wo")

        xtq = {}

        def x_tq(pre, x_d, c):
            xt = xy_p.tile(
                [128, KC * 1024], dt.bfloat16, tag="xy", name=f"x{pre}{c}"
            )[:, : KC * 512]
            nc.sync.dma_start(
                out=xt.rearrange("p (a i) -> p a i", a=KC),
                in_=x_d[:, c * 512 : (c + 1) * 512].rearrange(
                    "(a p) i -> p a i", p=128
                ),
            )
            xtq[pre, c] = xt

        def x_tq_half(pre, x_d, c, hf):
            """256-token half-quarter load (finer prologue pipelining)."""
            if (pre, c) not in xtq:
                xtq[pre, c] = xy_p.tile(
                    [128, KC * 1024], dt.bfloat16, tag="xy", name=f"x{pre}{c}"
                )[:, : KC * 512]
            nc.sync.dma_start(
                out=xtq[pre, c].rearrange("p (a i) -> p a i", a=KC)[
                    :, :, hf * 256 : (hf + 1) * 256
                ],
                in_=x_d[
                    :, c * 512 + hf * 256 : c * 512 + (hf + 1) * 256
                ].rearrange("(a p) i -> p a i", p=128),
            )

        def mask_cols(j0, nb, ph):
            """Load nb j-blocks of parity j0%2 starting at j0, cols half ph."""
            par = j0 % 2
            t, d, smap = (
                (mask_e, maskT8_d, E_SLOT) if par == 0
                else (mask_o, maskTb_d, O_SLOT)
            )
            slot = smap[j0, ph]
            nc.sync.dma_start(
                out=t.rearrange("p (b i) -> p b i", i=1024)[
                    :, slot : slot + nb, :
                ],
                in_=d.rearrange("(b two p) i -> p two b i", two=2, p=128)[
                    :, par, j0 // 2 : j0 // 2 + nb, ph * 1024 : (ph + 1) * 1024
                ],
            )

        # early odd-j (panel-0) mask slices staged fp8: they ride the fused
        # stt path, saving deadline-critical early DMA
        mo8 = cp.tile([128, 5 * 1024], f8, tag="mo8")
        # ph0 evens j0, j2, j4, j8 staged bf16: their tiles (k=0,2,4,8 and
        # 16,18,20,24) ride the tsp+Pool path in Pool's idle windows,
        # relieving DVE's 1127ns stt load where it drifts behind Act
        ME_SLOT = {0: 0, 2: 1, 4: 2, 8: 3}
        me16 = cp.tile([128, 4 * 1024], dt.bfloat16, tag="me16")

        def mask_slc(j, ph, h=0):
            if ph == 0 and j % 2 == 1 and j <= 9:
                return mo8[:, (j // 2) * 1024 : (j // 2 + 1) * 1024]
            if ph == 0 and j in ME_SLOT:
                s_ = ME_SLOT[j]
                return me16[:, s_ * 1024 : (s_ + 1) * 1024]
            if ph == 0 and h == 1 and j in (12, 14):
                # h1's copy of these slices is bf16 (Pool path); h0 uses fp8
                slot = O_SLOT[j, 0]
                return mask_o[:, slot * 1024 : (slot + 1) * 1024]
            t, smap = (mask_e, E_SLOT) if j % 2 == 0 else (mask_o, O_SLOT)
            slot = smap[j, ph]
            return t[:, slot * 1024 : (slot + 1) * 1024]

        vin = []

        def v_quarter(c):
            t_ = vin_p.tile(
                [128, KC * 512], dt.bfloat16, tag=f"vq{c % 3}", name=f"vq{c}"
            )
            nc.sync.dma_start(
                out=t_.rearrange("p (a i) -> p a i", a=KC),
                in_=vT_d[:, c * 512 : (c + 1) * 512].rearrange(
                    "(a p) i -> p a i", p=128
                ),
            )
            vin.append(t_)

        # DMA queue order: deadline-sorted (EDF) just-in-time stream over
        # the serial ~360GB/s transfer device. x-loads gate the S->exp
        # chain directly (hard deadlines); masks/v have pipeline slack.
        x_tq_half("k", kT_d, 0, 0)
        nc.sync.dma_start(out=wq_sb, in_=wq_d)
        nc.sync.dma_start(out=bqk_sb, in_=bqk_d)
        x_tq_half("q", qT_d, 0, 0)
        x_tq_half("q", qT_d, 0, 1)
        x_tq_half("q", qT_d, 1, 0)
        x_tq_half("q", qT_d, 1, 1)
        x_tq_half("k", kT_d, 0, 1)
        x_tq("k", kT_d, 1)
        x_tq_half("k", kT_d, 2, 0)
        x_tq_half("k", kT_d, 2, 1)
        nc.sync.dma_start(              # j0, j2 ph0 bf16 (Pool path)
            out=me16[:, 0 : 2 * 1024].rearrange("p (b i) -> p b i", i=1024),
            in_=maskTb_d.rearrange("(b two p) i -> p two b i", two=2, p=128)[
                :, 0, 0:2, 0:1024
            ],
        )
        nc.sync.dma_start(              # j1,3,5,7,9 ph0 as fp8
            out=mo8.rearrange("p (b i) -> p b i", i=1024),
            in_=maskT8_d.rearrange("(b two p) i -> p two b i", two=2, p=128)[
                :, 1, 0:5, 0:1024
            ],
        )
        x_tq_half("k", kT_d, 3, 0)
        x_tq_half("k", kT_d, 3, 1)
        nc.sync.dma_start(out=wv_sb, in_=wv_d)
        nc.sync.dma_start(              # j4 ph0 bf16 (Pool path)
            out=me16[:, 2 * 1024 : 3 * 1024],
            in_=maskTb_d.rearrange("(b two p) i -> p two b i", two=2, p=128)[
                :, 0, 2, 0:1024
            ],
        )
        mask_cols(6, 1, 0)              # j6 (fp8)
        v_quarter(0)
        nc.sync.dma_start(              # j8 ph0 bf16 (Pool path)
            out=me16[:, 3 * 1024 : 4 * 1024],
            in_=maskTb_d.rearrange("(b two p) i -> p two b i", two=2, p=128)[
                :, 0, 4, 0:1024
            ],
        )
        mask_cols(10, 1, 0)             # j10 (fp8)
        v_quarter(1)
        mask_cols(11, 2, 0)             # j11, j13 (bf16)
        mask_cols(12, 2, 0)             # j12, j14 (fp8)
        v_quarter(2)
        mask_cols(15, 1, 0)             # j15 (bf16)
        v_quarter(3)
        x_tq_half("q", qT_d, 2, 0)
        x_tq_half("q", qT_d, 2, 1)
        x_tq_half("q", qT_d, 3, 0)
        x_tq_half("q", qT_d, 3, 1)
        nc.sync.dma_start(              # j12, j14 ph0 bf16 copies (h1 Pool)
            out=mask_o.rearrange("p (b i) -> p b i", i=1024)[:, 11:13, :],
            in_=maskTb_d.rearrange("(b two p) i -> p two b i", two=2, p=128)[
                :, 0, 6:8, 0:1024
            ],
        )
        nc.sync.dma_start(out=cv_sb, in_=cv_d)
        mask_cols(0, 2, 1)              # ph1 evens j0, j2
        mask_cols(1, 2, 1)              # ph1 odds j1, j3
        mask_cols(4, 2, 1)
        mask_cols(5, 2, 1)
        nc.sync.dma_start(out=wo_sb, in_=wo_d)
        mask_cols(8, 2, 1)
        mask_cols(9, 2, 1)
        mask_cols(12, 2, 1)
        mask_cols(13, 2, 1)
        nc.sync.dma_start(out=ident, in_=id_d)

        ones2 = cp.tile([2, 128], dt.bfloat16, tag="ones2")
        nc.vector.memset(ones2, 1.0)

        # ---- attention: PSUM = s 2x2 + o 1x2 + y/v 2x1 = 8 banks ----
        with tc.tile_pool(name="ps_s", bufs=2, space="PSUM") as ps_p, \
             tc.tile_pool(name="ps_o", bufs=1, space="PSUM") as po_p, \
             tc.tile_pool(name="ps_v", bufs=2, space="PSUM") as pv_p:

            def v_proj_chunk(t):
                """Token-chunk t of the V projection into vaug."""
                ps = pv_p.tile([128, DPC], dt.float32, tag="vps", name=f"pv{t}")
                c, ts_ = divmod(t, 4)
                for kk in range(KC):
                    nc.tensor.matmul(
                        ps,
                        lhsT=vin[c][:, kk * 512 + ts_ * 128 : kk * 512 + (ts_ + 1) * 128],
                        rhs=wv_sb[:, kk * DPC : (kk + 1) * DPC],
                        start=(kk == 0),
                        stop=(kk == KC - 1),
                    )
                base = t * (HPC * VA)
                dst = vaug[:, base : base + HPC * VA].rearrange(
                    "p (a v) -> p a v", v=VA
                )[:, :, 0:DK]
                src = ps.rearrange("p (a d) -> p a d", d=DK)
                if t >= 12:
                    # late chunks evict on Act: DVE is the pacer by then
                    # and Act idles waiting on the panel-1 q stream
                    nc.scalar.activation(dst, src, AF.Copy)
                else:
                    nc.vector.tensor_copy(dst, src)

            def pv_mms(h, j, et, o_ps):
                for ic in range(IC):
                    nc.tensor.matmul(
                        o_ps[:, _oslc(ic) : _oslc(ic) + VA],
                        lhsT=et[:, ic * 128 : (ic + 1) * 128],
                        rhs=vaug[:, j * (HPC * VA) + h * VA : j * (HPC * VA) + (h + 1) * VA],
                        start=(j == 0 and ic % 7 == 0),
                        stop=False,
                    )

            def c_inject(h, o_ps):
                """+C (hi+lo rows); last slice per bank carries the stop."""
                for ic in range(IC):
                    nc.tensor.matmul(
                        o_ps[:, _oslc(ic) : _oslc(ic) + VA],
                        lhsT=ones2,
                        rhs=cv_sb[:, h * VA : (h + 1) * VA],
                        start=False,
                        stop=(ic in (6, 7)),
                    )

            import concourse.bass as bass_mod

            def norm_bank(p, h, o_ps, ot_pan, b, prio=False):
                """Normalize one PSUM bank of o_ps into ot_pan. With prio,
                schedule the DVE ops early in the stream: the norm releases
                the o-PSUM buffer the next head's PV (and thus the whole PE
                stream) waits on."""
                ctx2 = tc.high_priority(offset=NORM_PRIO) if prio else None
                if ctx2 is not None:
                    ctx2.__enter__()
                try:
                    n_ic = (7, 1)[b]
                    rc = rc_p.tile(
                        [128, 8], dt.float32, tag="rc", name=f"rc{p}_{h}_{b}"
                    )
                    den = bass_mod.AP(
                        tensor=o_ps.tensor,
                        offset=o_ps.offset + b * 512 + DK,
                        ap=[o_ps.ap[0], [VA, n_ic]],
                    )
                    nc.vector.reciprocal(rc[:, :n_ic], den)
                    src_ap = bass_mod.AP(
                        tensor=o_ps.tensor,
                        offset=o_ps.offset + b * 512,
                        ap=[o_ps.ap[0], [VA, n_ic], [1, DK]],
                    )
                    rcb = bass_mod.AP(
                        tensor=rc.tensor,
                        offset=rc.offset,
                        ap=[rc.ap[0], [1, n_ic], [0, DK]],
                    )
                    dst = bass_mod.AP(
                        tensor=ot_pan.tensor,
                        offset=ot_pan.offset + b * 7 * 128 + h * DK,
                        ap=[ot_pan.ap[0], [128, n_ic], [1, DK]],
                    )
                    nc.vector.tensor_mul(dst, src_ap, rcb)
                finally:
                    if ctx2 is not None:
                        ctx2.__exit__(None, None, None)

            def o_chunk(p, nn, half, eng, pool=None, tag="vps"):
                """One 512-col y chunk: matmul + eviction into y_pan."""
                y_ps = (pool or pv_p).tile(
                    [128, 512], dt.float32, tag=tag, name=f"y{p}_{nn}_{half}"
                )
                nc.tensor.matmul(
                    y_ps,
                    lhsT=wo_sb[:, nn * 128 : (nn + 1) * 128],
                    rhs=oT_full[:, p * 1024 + half * 512 : p * 1024 + (half + 1) * 512],
                    start=True,
                    stop=True,
                )
                eng.tensor_copy(
                    y_pan[p][:, nn * 1024 + half * 512 : nn * 1024 + (half + 1) * 512],
                    y_ps,
                )

            def y_dma(p, lo, hi):
                """DMA y_pan[p] rows nn in [lo,hi) out to yT."""
                nc.sync.dma_start(
                    out=yT_d[lo * 128 : hi * 128, p * 1024 : (p + 1) * 1024]
                    .rearrange("(a p2) i -> p2 a i", p2=128),
                    in_=y_pan[p][:, lo * 1024 : hi * 1024]
                    .rearrange("p (a i) -> p a i", i=1024),
                )

            # ---- flat 64-iteration pipeline ----
            ot_map = {}

            def emit_s(k):
                p, h, j = k // 32, (k // 16) % 2, k % 16
                hs = h * DK
                s_ps = ps_p.tile(
                    [128, 1024], dt.float32, tag="sps", name=f"s{k}"
                )
                with tc.high_priority():
                    for q in range(2):
                        nc.tensor.matmul(
                            s_ps[:, q * 512 : (q + 1) * 512],
                            lhsT=kT_sb[hs : hs + DK, j * 128 : (j + 1) * 128],
                            rhs=qT_sb[hs : hs + DK,
                                      p * 1024 + q * 512 : p * 1024 + (q + 1) * 512],
                            start=True,
                            stop=True,
                        )
                return s_ps

            def transposes(p):
                # batched xbar transposes, bank-0's 7 chunks first
                nc.sync.dma_start_transpose(
                    out=oT_full[:, p * 1024 : p * 1024 + 896].rearrange(
                        "p2 (b c) -> p2 b c", c=128
                    ),
                    in_=ot_map[p][:, 0:896],
                )
                nc.sync.dma_start_transpose(
                    out=oT_full[:, p * 1024 + 896 : (p + 1) * 1024],
                    in_=ot_map[p][:, 896:1024],
                )

            def norm_banks(p, h, o_ps, ot_pan):
                norm_bank(p, h, o_ps, ot_pan, 0)
                norm_bank(p, h, o_ps, ot_pan, 1)

            o_ps_map = {}
            pend = []
            dstate = {"pause": 0}

            def drain_one():
                pp, ph, pj, peh = pend.pop(0)
                if (pp, ph) not in o_ps_map:
                    o_ps_map[pp, ph] = po_p.tile(
                        [128, 1024], dt.float32, tag="ops", name=f"ops{pp}{ph}"
                    )
                o_ps = o_ps_map[pp, ph]
                pv_mms(ph, pj, peh, o_ps)
                if pj == JC - 1:
                    c_inject(ph, o_ps)
                    if (pp, ph) != (TP - 1, HPC - 1):
                        norm_banks(pp, ph, o_ps, ot_map[pp])
                        if ph == HPC - 1:
                            transposes(pp)
                        dstate["pause"] = HANDOFF_PAUSE

            def proj_panel(pre, c):
                w, dest = (wk_sb, kT_sb) if pre == "k" else (wq_sb, qT_sb)
                bcol = bqk_sb[:, 1:2] if pre == "k" else bqk_sb[:, 0:1]
                ps = pv_p.tile(
                    [128, 512], dt.float32, tag="vps", name=f"pp{pre}{c}"
                )
                for kk in range(KC):
                    nc.tensor.matmul(
                        ps,
                        lhsT=w[:, kk * DPC : (kk + 1) * DPC],
                        rhs=xtq[pre, c][:, kk * 512 : (kk + 1) * 512],
                        start=(kk == 0),
                        stop=(kk == KC - 1),
                    )
                # DVE eviction: an Act Identity here would displace an exp
                nc.vector.tensor_scalar_add(
                    dest[:, c * 512 : (c + 1) * 512], ps, bcol
                )

            def proj_panel_half(pre, c, hf, act=False):
                """256-token half-panel projection (prologue pipelining)."""
                w, dest = (wk_sb, kT_sb) if pre == "k" else (wq_sb, qT_sb)
                bcol = bqk_sb[:, 1:2] if pre == "k" else bqk_sb[:, 0:1]
                ps = pv_p.tile(
                    [128, 256], dt.float32, tag="vps", name=f"ph{pre}{c}{hf}"
                )
                lo = hf * 256
                for kk in range(KC):
                    nc.tensor.matmul(
                        ps,
                        lhsT=w[:, kk * DPC : (kk + 1) * DPC],
                        rhs=xtq[pre, c][:, kk * 512 + lo : kk * 512 + lo + 256],
                        start=(kk == 0),
                        stop=(kk == KC - 1),
                    )
                if act:
                    nc.scalar.activation(
                        dest[:, c * 512 + lo : c * 512 + lo + 256], ps,
                        AF.Identity, bias=bcol,
                    )
                else:
                    nc.vector.tensor_scalar_add(
                        dest[:, c * 512 + lo : c * 512 + lo + 256], ps, bcol
                    )

            proj_panel_half("k", 0, 0)
            proj_panel_half("q", 0, 0)
            proj_panel_half("q", 0, 1)
            # S(0)'s first half only needs qT cols 0-511: run it while the
            # xq1 halves are still streaming in
            s0 = ps_p.tile([128, 1024], dt.float32, tag="sps", name="s0")
            with tc.high_priority():
                nc.tensor.matmul(
                    s0[:, 0:512],
                    lhsT=kT_sb[0:DK, 0:128],
                    rhs=qT_sb[0:DK, 0:512],
                    start=True,
                    stop=True,
                )
            proj_panel_half("q", 1, 0)
            proj_panel_half("q", 1, 1)

            def p0_chunk(ck, eng):
                """One 512-col panel-0 y chunk; eviction on `eng`."""
                nn, half = ck // 2, ck % 2
                y_ps = pv_p.tile(
                    [128, 512], dt.float32, tag="vps", name=f"y0_{ck}"
                )
                nc.tensor.matmul(
                    y_ps,
                    lhsT=wo_sb[:, nn * 128 : (nn + 1) * 128],
                    rhs=oT_full[:, half * 512 : (half + 1) * 512],
                    start=True,
                    stop=True,
                )
                if eng is nc.scalar:
                    nc.scalar.activation(
                        y_pan[0][:, nn * 1024 + half * 512 :
                                 nn * 1024 + (half + 1) * 512],
                        y_ps, AF.Copy,
                    )
                else:
                    eng.tensor_copy(
                        y_pan[0][:, nn * 1024 + half * 512 :
                                 nn * 1024 + (half + 1) * 512],
                        y_ps,
                    )

            # per-iteration elementwise path:
            #  - fp8 stt (fused, 1x DVE): all even-j + panel-0 odd j<=9
            #  - tsp(e0-1) 4x + DVE 2x mult: tiles feeding the panel-0 norm
            #    chain (pull eh27-31 early) and the last tiles (Pool lags)
            #  - tsp(e0-1) 4x + Pool mult: everything else
            # DVE-mult tiles: the last odd-j of each head feed the norm ->
            # next-head-PV chain (o-PSUM buffer reuse); Pool's lag there
            # would stall the S stream at every head handoff
            DVEMUL_K = set(DVEMUL)

            def fp8_k(k):
                j = k % 16
                if k < 32 and j in (0, 2, 4, 8):
                    return False  # bf16, tsp+Pool in Pool's idle windows
                if 16 <= k < 32 and j in (12, 14):
                    return False  # dual-loaded: h1 copy is bf16 Pool path
                return j % 2 == 0 or (k < 32 and j <= 9)

            def depth(k):
                """PV pipeline depth: deep early (v-load slack), shallower
                mid (pulls the panel-0 norm chain ahead of its y consumers),
                tapering at the end so the post-loop backlog stays short."""
                if k < 30:
                    return EARLY_DEPTH
                if k < 52:
                    d = max(MID_DEPTH, EARLY_DEPTH - 2 * (k - 29))
                    if 36 <= k < 42:
                        d = min(d, DIP_DEPTH)
                    return d
                return max(END_DEPTH, MID_DEPTH - (k - 51))

            # S(0)'s second half, then the k0b half-projection
            with tc.high_priority():
                nc.tensor.matmul(
                    s0[:, 512:1024],
                    lhsT=kT_sb[0:DK, 0:128],
                    rhs=qT_sb[0:DK, 512:1024],
                    start=True,
                    stop=True,
                )
            s_next = s0
            proj_panel_half("k", 0, 1)
            for k in range(64):
                p, h, j = k // 32, (k // 16) % 2, k % 16
                if p not in ot_map:
                    ot_map[p] = otp_p.tile(
                        [128, IC * 128], dt.bfloat16, tag="otp", name=f"otp{p}"
                    )
                    y_pan[p] = xy_p.tile(
                        [128, KC * 1024], dt.bfloat16, tag="xy", name=f"ypan{p}"
                    )
                s_ps = s_next
                e0 = e_p.tile(
                    [128, 1024], dt.bfloat16, tag="e0", name=f"e0_{k}"
                )
                nc.scalar.activation(e0, s_ps, AF.Exp, scale=1.0 / math.sqrt(DK))
                eh = eh_p.tile(
                    [128, 1024], dt.bfloat16, tag="eh", name=f"eh{k}"
                )
                mslc = mask_slc(j, p, h)
                if fp8_k(k):
                    nc.vector.scalar_tensor_tensor(
                        eh, e0, 1.0, mslc, ALU.subtract, ALU.mult
                    )
                else:
                    t = t_p.tile(
                        [128, 1024], dt.bfloat16, tag="tm", name=f"tm{k}"
                    )
                    nc.vector.tensor_scalar_add(t, e0, -1.0)
                    if k in DVEMUL_K:
                        nc.vector.tensor_mul(eh, t, mslc)
                    else:
                        nc.gpsimd.tensor_mul(eh, t, mslc)
                # next S ahead of PV/side work so Act is never starved
                if k + 1 < 64:
                    s_next = emit_s(k + 1)
                # side work riding this iteration
                if k == 2:
                    proj_panel("k", 1)
                elif k == 6:
                    proj_panel_half("k", 2, 0)
                    proj_panel_half("k", 2, 1)
                elif k == 10:
                    proj_panel_half("k", 3, 0)
                    proj_panel_half("k", 3, 1)
                elif k == 28:
                    proj_panel_half("q", 2, 0)
                    proj_panel_half("q", 2, 1)
                elif k == 30:
                    proj_panel_half("q", 3, 0)
                    proj_panel_half("q", 3, 1)
                if 13 <= k <= 28:
                    v_proj_chunk(k - 13)
                # panel-0 y chunks ride the odd iterations of the second
                # half (oT_full panel 0 lands ~k=41); their DMAs go out in
                # row-pair groups as soon as both halves of a pair exist
                if P0_START <= k and k % 2 == 1:
                    ck = (k - P0_START) // 2
                    p0_chunk(ck, nc.vector)
                    if ck % 4 == 3:
                        y_dma(0, ck // 2 - 1, ck // 2 + 1)
                # variable-depth software pipeline for PV; after a head's
                # last j-block drains, pause 2 iterations so the norm ->
                # o-buffer-reuse chain overlaps the S stream instead of
                # stalling the next head's first PV
                if dstate["pause"] > 0:
                    dstate["pause"] -= 1
                else:
                    while len(pend) >= depth(k):
                        drain_one()
                        if dstate["pause"]:
                            break
                pend.append((p, h, j, eh))

            # remaining panel-0 y chunks: emitted before the PV flush so
            # their matmuls keep PE hot while the last PV/norm chain runs
            n_inb = max(0, (63 - P0_START) // 2 + 1)
            rows_dmad = 2 * sum(1 for c2 in range(n_inb) if c2 % 4 == 3)
            for ck in range(n_inb, 16):
                p0_chunk(ck, nc.vector if ck % 2 == 0 else nc.scalar)
                if ck % 2 == 1 and (ck + 1) // 2 - rows_dmad >= 2:
                    y_dma(0, rows_dmad, (ck + 1) // 2)
                    rows_dmad = (ck + 1) // 2
            if rows_dmad < 8:
                y_dma(0, rows_dmad, 8)

            while pend:
                drain_one()

            # ---- tail: panel-1 epilogue with PE transposes (PE and
            # all engines idle here; skips the 3us DMA-xbar latency) ----
            def y_dma_cols(p, half, lo, hi):
                nc.sync.dma_start(
                    out=yT_d[lo * 128 : hi * 128,
                             p * 1024 + half * 512 : p * 1024 + (half + 1) * 512]
                    .rearrange("(a p2) i -> p2 a i", p2=128),
                    in_=y_pan[p].rearrange("p (a i) -> p a i", i=1024)[
                        :, lo:hi, half * 512 : (half + 1) * 512
                    ],
                )

            o_ps = o_ps_map[TP - 1, HPC - 1]
            ot1 = ot_map[TP - 1]
            rr = (nc.vector, nc.scalar)

            def pe_transpose(lic):
                tp = ps_p.tile(
                    [128, 128], dt.bfloat16, tag="sps", name=f"tp{lic}"
                )
                nc.tensor.transpose(tp, ot1[:, lic * 128 : (lic + 1) * 128], ident)
                eng = nc.vector
                dst = oT_full[:, 1024 + lic * 128 : 1024 + (lic + 1) * 128]
                if eng is nc.scalar:
                    nc.scalar.activation(dst, tp, AF.Copy)
                else:
                    eng.tensor_copy(dst, tp)

            def tail_chunk(ck, half, nn=None):
                nn = ck % 8 if nn is None else nn
                eng = rr[ck % 2]
                pool, tag = (pv_p, "vps") if ck % 2 == 0 else (ps_p, "sps")
                if eng is nc.scalar:
                    y_ps = pool.tile(
                        [128, 512], dt.float32, tag=tag, name=f"y1_{ck}"
                    )
                    nc.tensor.matmul(
                        y_ps,
                        lhsT=wo_sb[:, nn * 128 : (nn + 1) * 128],
                        rhs=oT_full[:, 1024 + half * 512 : 1024 + (half + 1) * 512],
                        start=True,
                        stop=True,
                    )
                    nc.scalar.activation(
                        y_pan[1][:, nn * 1024 + half * 512 :
                                 nn * 1024 + (half + 1) * 512],
                        y_ps, AF.Copy,
                    )
                else:
                    o_chunk(1, nn, half, eng, pool=pool, tag=tag)

            norm_bank(TP - 1, HPC - 1, o_ps, ot1, 0, prio=True)
            for lic in range(7):
                pe_transpose(lic)
            for ck in range(8):
                tail_chunk(ck, 0)
                if ck == 3:
                    y_dma_cols(1, 0, 0, 4)
            y_dma_cols(1, 0, 4, 8)
            norm_bank(TP - 1, HPC - 1, o_ps, ot1, 1, prio=True)
            pe_transpose(7)
            for ck in range(8, 16):
                tail_chunk(ck, 1)
                if ck == 11:
                    y_dma_cols(1, 1, 0, 4)
                elif ck == 13:
                    y_dma_cols(1, 1, 4, 6)
                elif ck == 14:
                    y_dma_cols(1, 1, 6, 7)
            y_dma_cols(1, 1, 7, 8)

    nc.compile()
    return nc


def get_program():
    if "nc" not in _CACHE:
        _CACHE["nc"] = _build_program()
    return _CACHE["nc"]


def _wshuf(wT):
    """[1024 k, 128 n] -> [128 p, KC*128] with chunk kk at cols kk*128."""
    return np.ascontiguousarray(
        wT.reshape(KC, 128, DPC).transpose(1, 0, 2).reshape(128, KC * DPC)
    ).astype(BF16)


def make_in_maps(query, key, value, attention_mask, Wq, bq, Wk, bk, Wv, Wo):
    """Host-side sharding: per-core input dicts."""
    qT = np.ascontiguousarray(np.asarray(query, np.float32)[0].T).astype(BF16)
    kT = np.ascontiguousarray(np.asarray(key, np.float32)[0].T).astype(BF16)
    vT = np.ascontiguousarray(np.asarray(value, np.float32)[0].T).astype(BF16)
    maskTf = np.ascontiguousarray(np.asarray(attention_mask, np.float32)[0, 0].T)
    maskT8 = maskTf.astype(FP8)
    maskTb = maskTf.astype(BF16)
    # C = colsum(Vaug) per head = [colsum(value) @ Wv_h.T | S], fp64 on host,
    # split into bf16 hi+lo rows for near-fp32 injection accuracy
    vcol = np.asarray(value, np.float64)[0].sum(axis=0)  # [H]

    in_maps = []
    for c in range(NCORES):
        ns = slice(c * DPC, (c + 1) * DPC)
        cfull = vcol @ np.asarray(Wv, np.float64)[ns].T  # [DPC]
        cvec = np.zeros((2, HPC * VA), np.float64)
        for h in range(HPC):
            cvec[0, h * VA : h * VA + DK] = cfull[h * DK : (h + 1) * DK]
            cvec[0, h * VA + DK] = float(S)
        chi = cvec.astype(BF16)
        clo = (cvec - chi.astype(np.float64)).astype(BF16)
        cboth = np.concatenate([chi[0:1], clo[0:1]], axis=0)
        bqk = np.stack(
            [np.asarray(bq, np.float32)[ns], np.asarray(bk, np.float32)[ns]],
            axis=1,
        )
        in_maps.append(
            {
                "qT": qT,
                "kT": kT,
                "vT": vT,
                "maskT8": maskT8,
                "maskTb": maskTb,
                "wq": _wshuf(np.asarray(Wq, np.float32)[ns].T),
                "wk": _wshuf(np.asarray(Wk, np.float32)[ns].T),
                "wv": _wshuf(np.asarray(Wv, np.float32)[ns].T),
                "wo": np.ascontiguousarray(np.asarray(Wo, np.float32)[:, ns].T).astype(BF16),
                "bqk": np.ascontiguousarray(bqk),
                "cvec": cboth,
                "ident": np.eye(128, dtype=BF16),
            }
        )
    return in_maps


def combine_outputs(results, Wv_bias, Wo, bo):
    """Sum per-core partial yT's (bf16 -> fp32), add host-folded biases."""
    acc = np.zeros((H, S), np.float32)
    for r in results:
        acc += r["yT"].astype(np.float32)
    bias = np.asarray(bo, np.float32) + np.asarray(Wv_bias, np.float32) @ np.asarray(
        Wo, np.float32
    ).T
    return (acc.T + bias[None, :]).astype(np.float32)[None]


def kernel(
    query,
    key,
    value,
    attention_mask,
    Wq,
    bq,
    Wk,
    bk,
    Wv,
    bv,
    Wo,
    bo,
    head,
    hidden_size,
):
    from concourse.bass_utils import run_bass_kernel_spmd

    nc = get_program()
    in_maps = make_in_maps(
        query, key, value, attention_mask, Wq, bq, Wk, bk, Wv, Wo
    )
    res = run_bass_kernel_spmd(nc, in_maps, list(range(NCORES)))
    return combine_outputs(res.results, bv, Wo, bo)



# revision 125
# speedup vs baseline: 1.0082x; 1.0050x over previous
"""Multi-head attention (B=1, S=2048, H=1024, NH=16) on 8 trn2 NeuronCores.

Sharding: head-parallel. Core c owns heads {2c, 2c+1} (= 128 of the 1024
hidden dims). Each core computes its Q/K/V projection slices, the full
attention for its 2 heads, and a full-width partial of the output
projection (contraction over its 128 context dims). Host sums the 8
partials and adds the (host-folded) biases.

Attention elementwise path (the reference quirk: masked scores are set
to 0 pre-softmax, so masked lanes contribute exp(0)=1):

    E = m*exp(s/8) + (1-m)            (m in {0,1})
      = m*(e0 - 1) + 1,   e0 = exp(s/8)

  * Act engine: e0 = Exp(s_psum / 8) straight out of PSUM -> SBUF bf16,
    one 1024-col tile per iteration; Act is the body's rate limiter
    (64 x 1038 ns) and does ~nothing else until the epilogue.
  * Masked-combine, one of three per-tile paths (Pool has no PSUM
    access; fused stt runs 1x on DVE; plain tensor_scalar/tensor_tensor
    hit DVE 4x/2x modes with all-bf16 SBUF operands):
      - fp8 stt (1127ns DVE): fused (e0-1)*m, fp8 mask - most evens +
        panel-0 odd j<=9.
      - tsp+Pool: t = e0 - 1 on DVE at 4x (327ns), eh = t*m on Pool
        (2.2us, deep pipeline slack) - tiles placed in Pool's idle
        windows: ph0-even j0/2/4/8 (k=0..24) and all ph1 odds. ph0
        j12/j14 are dual-loaded (fp8 for h0, bf16 copy for h1) so h1's
        tiles ride Pool where DVE otherwise drifts behind Act.
      - tsp+DVE-mult (921ns): tiles feeding each head-handoff's norm
        chain (k=27-31) and the last tiles (61,63), where Pool's queue
        latency would stall the o-buffer handoff or the tail.
    Load-balance invariant: DVE+Act carry exp (66.4us) + combines +
    evictions ~ 146us between them; every avoidable DVE ns matters
    because Act can run only e_p=8 exps ahead (e0-buffer WAR), so mask
    lateness or DVE drift surfaces directly as Act stalls.
  * The "+1" term: sum_j 1*vaug[j,:] = colsum(Vaug) = C, an i-independent
    vector, injected into each PV PSUM accumulation as a single K=2
    matmul against host-precomputed C split into bf16 hi+lo rows.

Loop structure: one flat 64-iteration pipeline over (panel, head,
j-block) with panels of 1024 queries. S-matmuls are emitted one
iteration ahead (priority-0). PV pipeline depth: 14 while V streams
(k<30), dipping to 8 at k=36-41 to pull the p0h1->p1h0 norm handoff
ahead of its consumers, 9 mid, tapering to 3 at the end. V-projection
chunks ride k=13..28. Panel-0's y-chunks ride odd iterations k>=43
(oT panel 0 lands ~k=41 via xbar-transposes), their DMAs in row-pair
groups; 5 leftover chunks go just before the PV flush. Panel-1's y is
the tail: norm -> PE+identity transposes -> 16 chunks on DVE/Act with
column-half DMAs, last rows as 1-row DMAs. PSUM: s 2x2 + o 2 + y/v
2x1 = 8 banks.

DMA discipline (~360 GB/s serial transfer device, one HWDGE slot per
dma_start): the queue is deadline-sorted (EDF). Hard deadlines: x
loads gate proj->S->exp directly - prologue streams wk/wq + xk0/xq0/
xq1 as 256-token half-quarters chased by half-panel projections
(first exp ~13.5us), then xk1..3, then v quarters (PE-blocking via
the v-proj window). Soft deadlines: stt masks are due t(k + e_p) via
the e0-WAR; Pool-path masks only gate PV (t(k + depth)) and load in
the post-xq3 stream. Masks live in packed 1024-col slot tiles
(fp8 mask_e / bf16 mask_o + mo8/me16), y leaves in 0.25-0.5MB groups
as chunks complete. q/k biases fold into projection evictions.

Precision: identical to the reference-faithful baseline - all matmuls
bf16 with fp32 PSUM accumulation, softmax without max-subtraction
(exponent ~ N(0,0.33^2) cannot overflow). Modeled 110.5us (was 112.6);
hw rel err 1.4e-3.
"""

import math

import numpy as np
import ml_dtypes

BF16 = ml_dtypes.bfloat16
FP8 = ml_dtypes.float8_e4m3
S, H, NH, DK = 2048, 1024, 16, 64
NCORES = 8
HPC = NH // NCORES          # heads per core = 2
DPC = HPC * DK              # head dims per core = 128
KC = H // 128               # contraction chunks = 8
TP = 2                      # 1024-wide query token panels
JC = S // 128               # 128-wide key chunks = 16
IC = 1024 // 128            # i-chunks per panel = 8
VA = DK + 1                 # v columns + ones column = 65

_CACHE = {}

# schedule knobs (tuned via TimelineSim sweeps)
EARLY_DEPTH = 14     # PV pipeline depth while v-quarters stream
MID_DEPTH = 9        # depth after the early taper
END_DEPTH = 3        # post-loop drain backlog
P0_START = 43        # first in-body panel-0 y-chunk iteration (odd)
DVEMUL = (27, 29, 31, 61, 63)  # odd-j tiles multiplied on DVE
HANDOFF_PAUSE = 2    # iterations to pause draining after a head's last j
DIP_DEPTH = 8        # temporary depth dip at k=36-41 (handoff pull-in)
NORM_PRIO = 15     # priority offset for head-norm DVE ops (o-WAR release)


def _oslc(ic):
    """o_ps column offset for ic-th 65-wide slice: 7 slices in bank 0,
    the 8th at 512 so no matmul crosses a PSUM bank boundary."""
    b, r = divmod(ic, 7)
    return b * 512 + r * VA


def _build_program():
    """Build + compile the (identical) per-core Bass program."""
    from contextlib import ExitStack

    import concourse.bacc as bacc
    import concourse.tile as tile
    from concourse import mybir

    dt = mybir.dt
    AF = mybir.ActivationFunctionType
    ALU = mybir.AluOpType
    f8 = dt.float8e4

    nc = bacc.Bacc("TRN2", target_bir_lowering=False, debug=False)

    qT_d = nc.dram_tensor("qT", [H, S], dt.bfloat16, kind="ExternalInput").ap()
    kT_d = nc.dram_tensor("kT", [H, S], dt.bfloat16, kind="ExternalInput").ap()
    vT_d = nc.dram_tensor("vT", [H, S], dt.bfloat16, kind="ExternalInput").ap()
    maskT8_d = nc.dram_tensor("maskT8", [S, S], f8, kind="ExternalInput").ap()
    maskTb_d = nc.dram_tensor("maskTb", [S, S], dt.bfloat16, kind="ExternalInput").ap()
    wk_d = nc.dram_tensor("wk", [128, KC * DPC], dt.bfloat16, kind="ExternalInput").ap()
    wq_d = nc.dram_tensor("wq", [128, KC * DPC], dt.bfloat16, kind="ExternalInput").ap()
    wv_d = nc.dram_tensor("wv", [128, KC * DPC], dt.bfloat16, kind="ExternalInput").ap()
    wo_d = nc.dram_tensor("wo", [DPC, H], dt.bfloat16, kind="ExternalInput").ap()
    bqk_d = nc.dram_tensor("bqk", [128, 2], dt.float32, kind="ExternalInput").ap()
    cv_d = nc.dram_tensor("cvec", [2, HPC * VA], dt.bfloat16, kind="ExternalInput").ap()
    id_d = nc.dram_tensor("ident", [128, 128], dt.bfloat16, kind="ExternalInput").ap()
    yT_d = nc.dram_tensor("yT", [H, S], dt.bfloat16, kind="ExternalOutput").ap()

    with tile.TileContext(nc) as tc, ExitStack() as ctx:
        cp = ctx.enter_context(tc.tile_pool(name="const", bufs=1))
        e_p = ctx.enter_context(tc.tile_pool(name="ex", bufs=8))
        eh_p = ctx.enter_context(tc.tile_pool(name="ehat", bufs=15))
        rc_p = ctx.enter_context(tc.tile_pool(name="recip", bufs=2))
        t_p = ctx.enter_context(tc.tile_pool(name="tmul", bufs=3))
        mh_p = ctx.enter_context(tc.tile_pool(name="maskhi", bufs=1))
        otp_p = ctx.enter_context(tc.tile_pool(name="otpan", bufs=2))
        vin_p = ctx.enter_context(tc.tile_pool(name="vin", bufs=1))
        xy_p = ctx.enter_context(tc.tile_pool(name="xy", bufs=3))

        # ---- DMA schedule: wk | xk quarters | wq | xq quarters | rest ----
        wk_sb = cp.tile([128, KC * DPC], dt.bfloat16, tag="wk")
        nc.sync.dma_start(out=wk_sb, in_=wk_d)
        # preload the Exp activation table off the critical path
        warm = cp.tile([1, 2], dt.bfloat16, tag="warm")
        nc.vector.memset(warm, 0.0)
        nc.scalar.activation(warm, warm, AF.Exp)

        # ---- token-streamed inputs: x loads in token quarters so each
        # kT/qT panel completes as its quarter lands; S(j) needs only
        # kT token-block j and qT's active panel half, so attention
        # starts ~15us earlier. Late panels project as body side-work.
        wq_sb = cp.tile([128, KC * DPC], dt.bfloat16, tag="wq")
        bqk_sb = cp.tile([128, 2], dt.float32, tag="bqk")
        cv_sb = cp.tile([2, HPC * VA], dt.bfloat16, tag="cv")
        ident = cp.tile([128, 128], dt.bfloat16, tag="ident")
        qT_sb = cp.tile([128, S], dt.bfloat16, tag="qTs")
        kT_sb = cp.tile([128, S], dt.bfloat16, tag="kTs")
        vaug = cp.tile([128, JC * (HPC * VA)], dt.bfloat16, tag="vaug")
        nc.gpsimd.memset(
            vaug.rearrange("p (a v) -> p a v", v=VA)[:, :, DK:VA], 1.0
        )
        ot_pan = None
        oT_full = cp.tile([128, S], dt.bfloat16, tag="oTfull")
        y_pan = {}
        # packed mask slot layouts (1024-col slots), only the (j, ph)
        # combinations actually consumed from each dtype
        E_SLOT = {(6, 0): 0, (10, 0): 1, (12, 0): 2,
                  (14, 0): 3, (0, 1): 4, (2, 1): 5, (4, 1): 6, (6, 1): 7,
                  (8, 1): 8, (10, 1): 9, (12, 1): 10, (14, 1): 11}
        O_SLOT = {(11, 0): 0, (13, 0): 1, (15, 0): 2, (1, 1): 3, (3, 1): 4,
                  (5, 1): 5, (7, 1): 6, (9, 1): 7, (11, 1): 8, (13, 1): 9,
                  (15, 1): 10, (12, 0): 11, (14, 0): 12}
        mask_e = cp.tile([128, 12 * 1024], f8, tag="maske")
        mask_o = mh_p.tile([128, 13 * 1024], dt.bfloat16, tag="masko")
        wv_sb = cp.tile([128, KC * DPC], dt.bfloat16, tag="wv")
        wo_sb = cp.tile([128, H], dt.bfloat16, tag="wo")

        xtq = {}

        def x_tq(pre, x_d, c):
            xt = xy_p.tile(
                [128, KC * 1024], dt.bfloat16, tag="xy", name=f"x{pre}{c}"
            )[:, : KC * 512]
            nc.sync.dma_start(
                out=xt.rearrange("p (a i) -> p a i", a=KC),
                in_=x_d[:, c * 512 : (c + 1) * 512].rearrange(
                    "(a p) i -> p a i", p=128
                ),
            )
            xtq[pre, c] = xt

        def x_tq_half(pre, x_d, c, hf):
            """256-token half-quarter load (finer prologue pipelining)."""
            if (pre, c) not in xtq:
                xtq[pre, c] = xy_p.tile(
                    [128, KC * 1024], dt.bfloat16, tag="xy", name=f"x{pre}{c}"
                )[:, : KC * 512]
            nc.sync.dma_start(
                out=xtq[pre, c].rearrange("p (a i) -> p a i", a=KC)[
                    :, :, hf * 256 : (hf + 1) * 256
                ],
                in_=x_d[
                    :, c * 512 + hf * 256 : c * 512 + (hf + 1) * 256
                ].rearrange("(a p) i -> p a i", p=128),
            )

        def mask_cols(j0, nb, ph):
            """Load nb j-blocks of parity j0%2 starting at j0, cols half ph."""
            par = j0 % 2
            t, d, smap = (
                (mask_e, maskT8_d, E_SLOT) if par == 0
                else (mask_o, maskTb_d, O_SLOT)
            )
            slot = smap[j0, ph]
            nc.sync.dma_start(
                out=t.rearrange("p (b i) -> p b i", i=1024)[
                    :, slot : slot + nb, :
                ],
                in_=d.rearrange("(b two p) i -> p two b i", two=2, p=128)[
                    :, par, j0 // 2 : j0 // 2 + nb, ph * 1024 : (ph + 1) * 1024
                ],
            )

        # early odd-j (panel-0) mask slices staged fp8: they ride the fused
        # stt path, saving deadline-critical early DMA
        mo8 = cp.tile([128, 5 * 1024], f8, tag="mo8")
        # ph0 evens j0, j2, j4, j8 staged bf16: their tiles (k=0,2,4,8 and
        # 16,18,20,24) ride the tsp+Pool path in Pool's idle windows,
        # relieving DVE's 1127ns stt load where it drifts behind Act
        ME_SLOT = {0: 0, 2: 1, 4: 2, 8: 3}
        me16 = cp.tile([128, 4 * 1024], dt.bfloat16, tag="me16")

        def mask_slc(j, ph, h=0):
            if ph == 0 and j % 2 == 1 and j <= 9:
                return mo8[:, (j // 2) * 1024 : (j // 2 + 1) * 1024]
            if ph == 0 and j in ME_SLOT:
                s_ = ME_SLOT[j]
                return me16[:, s_ * 1024 : (s_ + 1) * 1024]
            if ph == 0 and h == 1 and j in (12, 14):
                # h1's copy of these slices is bf16 (Pool path); h0 uses fp8
                slot = O_SLOT[j, 0]
                return mask_o[:, slot * 1024 : (slot + 1) * 1024]
            t, smap = (mask_e, E_SLOT) if j % 2 == 0 else (mask_o, O_SLOT)
            slot = smap[j, ph]
            return t[:, slot * 1024 : (slot + 1) * 1024]

        vin = []

        def v_quarter(c):
            t_ = vin_p.tile(
                [128, KC * 512], dt.bfloat16, tag=f"vq{c % 3}", name=f"vq{c}"
            )
            nc.sync.dma_start(
                out=t_.rearrange("p (a i) -> p a i", a=KC),
                in_=vT_d[:, c * 512 : (c + 1) * 512].rearrange(
                    "(a p) i -> p a i", p=128
                ),
            )
            vin.append(t_)

        # DMA queue order: deadline-sorted (EDF) just-in-time stream over
        # the serial ~360GB/s transfer device. x-loads gate the S->exp
        # chain directly (hard deadlines); masks/v have pipeline slack.
        x_tq_half("k", kT_d, 0, 0)
        nc.sync.dma_start(out=wq_sb, in_=wq_d)
        nc.sync.dma_start(out=bqk_sb, in_=bqk_d)
        x_tq_half("q", qT_d, 0, 0)
        x_tq_half("q", qT_d, 0, 1)
        x_tq_half("q", qT_d, 1, 0)
        x_tq_half("q", qT_d, 1, 1)
        x_tq_half("k", kT_d, 0, 1)
        x_tq("k", kT_d, 1)
        x_tq_half("k", kT_d, 2, 0)
        x_tq_half("k", kT_d, 2, 1)
        nc.sync.dma_start(              # j0, j2 ph0 bf16 (Pool path)
            out=me16[:, 0 : 2 * 1024].rearrange("p (b i) -> p b i", i=1024),
            in_=maskTb_d.rearrange("(b two p) i -> p two b i", two=2, p=128)[
                :, 0, 0:2, 0:1024
            ],
        )
        nc.sync.dma_start(              # j1,3,5,7,9 ph0 as fp8
            out=mo8.rearrange("p (b i) -> p b i", i=1024),
            in_=maskT8_d.rearrange("(b two p) i -> p two b i", two=2, p=128)[
                :, 1, 0:5, 0:1024
            ],
        )
        x_tq_half("k", kT_d, 3, 0)
        x_tq_half("k", kT_d, 3, 1)
        nc.sync.dma_start(out=wv_sb, in_=wv_d)
        nc.sync.dma_start(              # j4 ph0 bf16 (Pool path)
            out=me16[:, 2 * 1024 : 3 * 1024],
            in_=maskTb_d.rearrange("(b two p) i -> p two b i", two=2, p=128)[
                :, 0, 2, 0:1024
            ],
        )
        mask_cols(6, 1, 0)              # j6 (fp8)
        v_quarter(0)
        nc.sync.dma_start(              # j8 ph0 bf16 (Pool path)
            out=me16[:, 3 * 1024 : 4 * 1024],
            in_=maskTb_d.rearrange("(b two p) i -> p two b i", two=2, p=128)[
                :, 0, 4, 0:1024
            ],
        )
        mask_cols(10, 1, 0)             # j10 (fp8)
        v_quarter(1)
        mask_cols(11, 2, 0)             # j11, j13 (bf16)
        mask_cols(12, 2, 0)             # j12, j14 (fp8)
        v_quarter(2)
        mask_cols(15, 1, 0)             # j15 (bf16)
        v_quarter(3)
        x_tq_half("q", qT_d, 2, 0)
        x_tq_half("q", qT_d, 2, 1)
        x_tq_half("q", qT_d, 3, 0)
        x_tq_half("q", qT_d, 3, 1)
        nc.sync.dma_start(              # j12, j14 ph0 bf16 copies (h1 Pool)
            out=mask_o.rearrange("p (b i) -> p b i", i=1024)[:, 11:13, :],
            in_=maskTb_d.rearrange("(b two p) i -> p two b i", two=2, p=128)[
                :, 0, 6:8, 0:1024
            ],
        )
        nc.sync.dma_start(out=cv_sb, in_=cv_d)
        mask_cols(0, 2, 1)              # ph1 evens j0, j2
        mask_cols(1, 2, 1)              # ph1 odds j1, j3
        mask_cols(4, 2, 1)
        mask_cols(5, 2, 1)
        nc.sync.dma_start(out=wo_sb, in_=wo_d)
        mask_cols(8, 2, 1)
        mask_cols(9, 2, 1)
        mask_cols(12, 2, 1)
        mask_cols(13, 2, 1)
        nc.sync.dma_start(out=ident, in_=id_d)

        ones2 = cp.tile([2, 128], dt.bfloat16, tag="ones2")
        nc.vector.memset(ones2, 1.0)

        # ---- attention: PSUM = s 2x2 + o 1x2 + y/v 2x1 = 8 banks ----
        with tc.tile_pool(name="ps_s", bufs=2, space="PSUM") as ps_p, \
             tc.tile_pool(name="ps_o", bufs=1, space="PSUM") as po_p, \
             tc.tile_pool(name="ps_v", bufs=2, space="PSUM") as pv_p:

            def v_proj_chunk(t):
                """Token-chunk t of the V projection into vaug."""
                ps = pv_p.tile([128, DPC], dt.float32, tag="vps", name=f"pv{t}")
                c, ts_ = divmod(t, 4)
                for kk in range(KC):
                    nc.tensor.matmul(
                        ps,
                        lhsT=vin[c][:, kk * 512 + ts_ * 128 : kk * 512 + (ts_ + 1) * 128],
                        rhs=wv_sb[:, kk * DPC : (kk + 1) * DPC],
                        start=(kk == 0),
                        stop=(kk == KC - 1),
                    )
                base = t * (HPC * VA)
                dst = vaug[:, base : base + HPC * VA].rearrange(
                    "p (a v) -> p a v", v=VA
                )[:, :, 0:DK]
                src = ps.rearrange("p (a d) -> p a d", d=DK)
                if t >= 12:
                    # late chunks evict on Act: DVE is the pacer by then
                    # and Act idles waiting on the panel-1 q stream
                    nc.scalar.activation(dst, src, AF.Copy)
                else:
                    nc.vector.tensor_copy(dst, src)

            def pv_mms(h, j, et, o_ps):
                for ic in range(IC):
                    nc.tensor.matmul(
                        o_ps[:, _oslc(ic) : _oslc(ic) + VA],
                        lhsT=et[:, ic * 128 : (ic + 1) * 128],
                        rhs=vaug[:, j * (HPC * VA) + h * VA : j * (HPC * VA) + (h + 1) * VA],
                        start=(j == 0 and ic % 7 == 0),
                        stop=False,
                    )

            def c_inject(h, o_ps):
                """+C (hi+lo rows); last slice per bank carries the stop."""
                for ic in range(IC):
                    nc.tensor.matmul(
                        o_ps[:, _oslc(ic) : _oslc(ic) + VA],
                        lhsT=ones2,
                        rhs=cv_sb[:, h * VA : (h + 1) * VA],
                        start=False,
                        stop=(ic in (6, 7)),
                    )

            import concourse.bass as bass_mod

            def norm_bank(p, h, o_ps, ot_pan, b, prio=False):
                """Normalize one PSUM bank of o_ps into ot_pan. With prio,
                schedule the DVE ops early in the stream: the norm releases
                the o-PSUM buffer the next head's PV (and thus the whole PE
                stream) waits on."""
                ctx2 = tc.high_priority(offset=NORM_PRIO) if prio else None
                if ctx2 is not None:
                    ctx2.__enter__()
                try:
                    n_ic = (7, 1)[b]
                    rc = rc_p.tile(
                        [128, 8], dt.float32, tag="rc", name=f"rc{p}_{h}_{b}"
                    )
                    den = bass_mod.AP(
                        tensor=o_ps.tensor,
                        offset=o_ps.offset + b * 512 + DK,
                        ap=[o_ps.ap[0], [VA, n_ic]],
                    )
                    nc.vector.reciprocal(rc[:, :n_ic], den)
                    src_ap = bass_mod.AP(
                        tensor=o_ps.tensor,
                        offset=o_ps.offset + b * 512,
                        ap=[o_ps.ap[0], [VA, n_ic], [1, DK]],
                    )
                    rcb = bass_mod.AP(
                        tensor=rc.tensor,
                        offset=rc.offset,
                        ap=[rc.ap[0], [1, n_ic], [0, DK]],
                    )
                    dst = bass_mod.AP(
                        tensor=ot_pan.tensor,
                        offset=ot_pan.offset + b * 7 * 128 + h * DK,
                        ap=[ot_pan.ap[0], [128, n_ic], [1, DK]],
                    )
                    nc.vector.tensor_mul(dst, src_ap, rcb)
                finally:
                    if ctx2 is not None:
                        ctx2.__exit__(None, None, None)

            def o_chunk(p, nn, half, eng, pool=None, tag="vps"):
                """One 512-col y chunk: matmul + eviction into y_pan."""
                y_ps = (pool or pv_p).tile(
                    [128, 512], dt.float32, tag=tag, name=f"y{p}_{nn}_{half}"
                )
                nc.tensor.matmul(
                    y_ps,
                    lhsT=wo_sb[:, nn * 128 : (nn + 1) * 128],
                    rhs=oT_full[:, p * 1024 + half * 512 : p * 1024 + (half + 1) * 512],
                    start=True,
                    stop=True,
                )
                eng.tensor_copy(
                    y_pan[p][:, nn * 1024 + half * 512 : nn * 1024 + (half + 1) * 512],
                    y_ps,
                )

            def y_dma(p, lo, hi):
                """DMA y_pan[p] rows nn in [lo,hi) out to yT."""
                nc.sync.dma_start(
                    out=yT_d[lo * 128 : hi * 128, p * 1024 : (p + 1) * 1024]
                    .rearrange("(a p2) i -> p2 a i", p2=128),
                    in_=y_pan[p][:, lo * 1024 : hi * 1024]
                    .rearrange("p (a i) -> p a i", i=1024),
                )

            # ---- flat 64-iteration pipeline ----
            ot_map = {}

            def emit_s(k):
                p, h, j = k // 32, (k // 16) % 2, k % 16
                hs = h * DK
                s_ps = ps_p.tile(
                    [128, 1024], dt.float32, tag="sps", name=f"s{k}"
                )
                with tc.high_priority():
                    for q in range(2):
                        nc.tensor.matmul(
                            s_ps[:, q * 512 : (q + 1) * 512],
                            lhsT=kT_sb[hs : hs + DK, j * 128 : (j + 1) * 128],
                            rhs=qT_sb[hs : hs + DK,
                                      p * 1024 + q * 512 : p * 1024 + (q + 1) * 512],
                            start=True,
                            stop=True,
                        )
                return s_ps

            def transposes(p):
                # batched xbar transposes, bank-0's 7 chunks first
                nc.sync.dma_start_transpose(
                    out=oT_full[:, p * 1024 : p * 1024 + 896].rearrange(
                        "p2 (b c) -> p2 b c", c=128
                    ),
                    in_=ot_map[p][:, 0:896],
                )
                nc.sync.dma_start_transpose(
                    out=oT_full[:, p * 1024 + 896 : (p + 1) * 1024],
                    in_=ot_map[p][:, 896:1024],
                )

            def norm_banks(p, h, o_ps, ot_pan):
                norm_bank(p, h, o_ps, ot_pan, 0)
                norm_bank(p, h, o_ps, ot_pan, 1)

            o_ps_map = {}
            pend = []
            dstate = {"pause": 0}

            def drain_one():
                pp, ph, pj, peh = pend.pop(0)
                if (pp, ph) not in o_ps_map:
                    o_ps_map[pp, ph] = po_p.tile(
                        [128, 1024], dt.float32, tag="ops", name=f"ops{pp}{ph}"
                    )
                o_ps = o_ps_map[pp, ph]
                pv_mms(ph, pj, peh, o_ps)
                if pj == JC - 1:
                    c_inject(ph, o_ps)
                    if (pp, ph) != (TP - 1, HPC - 1):
                        norm_banks(pp, ph, o_ps, ot_map[pp])
                        if ph == HPC - 1:
                            transposes(pp)
                        dstate["pause"] = HANDOFF_PAUSE

            def proj_panel(pre, c):
                w, dest = (wk_sb, kT_sb) if pre == "k" else (wq_sb, qT_sb)
                bcol = bqk_sb[:, 1:2] if pre == "k" else bqk_sb[:, 0:1]
                ps = pv_p.tile(
                    [128, 512], dt.float32, tag="vps", name=f"pp{pre}{c}"
                )
                for kk in range(KC):
                    nc.tensor.matmul(
                        ps,
                        lhsT=w[:, kk * DPC : (kk + 1) * DPC],
                        rhs=xtq[pre, c][:, kk * 512 : (kk + 1) * 512],
                        start=(kk == 0),
                        stop=(kk == KC - 1),
                    )
                # DVE eviction: an Act Identity here would displace an exp
                nc.vector.tensor_scalar_add(
                    dest[:, c * 512 : (c + 1) * 512], ps, bcol
                )

            def proj_panel_half(pre, c, hf, act=False):
                """256-token half-panel projection (prologue pipelining)."""
                w, dest = (wk_sb, kT_sb) if pre == "k" else (wq_sb, qT_sb)
                bcol = bqk_sb[:, 1:2] if pre == "k" else bqk_sb[:, 0:1]
                ps = pv_p.tile(
                    [128, 256], dt.float32, tag="vps", name=f"ph{pre}{c}{hf}"
                )
                lo = hf * 256
                for kk in range(KC):
                    nc.tensor.matmul(
                        ps,
                        lhsT=w[:, kk * DPC : (kk + 1) * DPC],
                        rhs=xtq[pre, c][:, kk * 512 + lo : kk * 512 + lo + 256],
                        start=(kk == 0),
                        stop=(kk == KC - 1),
                    )
                if act:
                    nc.scalar.activation(
                        dest[:, c * 512 + lo : c * 512 + lo + 256], ps,
                        AF.Identity, bias=bcol,
                    )
                else:
                    nc.vector.tensor_scalar_add(
                        dest[:, c * 512 + lo : c * 512 + lo + 256], ps, bcol
                    )

            proj_panel_half("k", 0, 0)
            proj_panel_half("q", 0, 0)
            proj_panel_half("q", 0, 1)
            # S(0)'s first half only needs qT cols 0-511: run it while the
            # xq1 halves are still streaming in
            s0 = ps_p.tile([128, 1024], dt.float32, tag="sps", name="s0")
            with tc.high_priority():
                nc.tensor.matmul(
                    s0[:, 0:512],
                    lhsT=kT_sb[0:DK, 0:128],
                    rhs=qT_sb[0:DK, 0:512],
                    start=True,
                    stop=True,
                )
            proj_panel_half("q", 1, 0)
            proj_panel_half("q", 1, 1)

            def p0_chunk(ck, eng):
                """One 512-col panel-0 y chunk; eviction on `eng`."""
                nn, half = ck // 2, ck % 2
                y_ps = pv_p.tile(
                    [128, 512], dt.float32, tag="vps", name=f"y0_{ck}"
                )
                nc.tensor.matmul(
                    y_ps,
                    lhsT=wo_sb[:, nn * 128 : (nn + 1) * 128],
                    rhs=oT_full[:, half * 512 : (half + 1) * 512],
                    start=True,
                    stop=True,
                )
                if eng is nc.scalar:
                    nc.scalar.activation(
                        y_pan[0][:, nn * 1024 + half * 512 :
                                 nn * 1024 + (half + 1) * 512],
                        y_ps, AF.Copy,
                    )
                else:
                    eng.tensor_copy(
                        y_pan[0][:, nn * 1024 + half * 512 :
                                 nn * 1024 + (half + 1) * 512],
                        y_ps,
                    )

            # per-iteration elementwise path:
            #  - fp8 stt (fused, 1x DVE): all even-j + panel-0 odd j<=9
            #  - tsp(e0-1) 4x + DVE 2x mult: tiles feeding the panel-0 norm
            #    chain (pull eh27-31 early) and the last tiles (Pool lags)
            #  - tsp(e0-1) 4x + Pool mult: everything else
            # DVE-mult tiles: the last odd-j of each head feed the norm ->
            # next-head-PV chain (o-PSUM buffer reuse); Pool's lag there
            # would stall the S stream at every head handoff
            DVEMUL_K = set(DVEMUL)

            def fp8_k(k):
                j = k % 16
                if k < 32 and j in (0, 2, 4, 8):
                    return False  # bf16, tsp+Pool in Pool's idle windows
                if 16 <= k < 32 and j in (12, 14):
                    return False  # dual-loaded: h1 copy is bf16 Pool path
                return j % 2 == 0 or (k < 32 and j <= 9)

            def depth(k):
                """PV pipeline depth: deep early (v-load slack), shallower
                mid (pulls the panel-0 norm chain ahead of its y consumers),
                tapering at the end so the post-loop backlog stays short."""
                if k < 30:
                    return EARLY_DEPTH
                if k < 52:
                    d = max(MID_DEPTH, EARLY_DEPTH - 2 * (k - 29))
                    if 36 <= k < 42:
                        d = min(d, DIP_DEPTH)
                    return d
                return max(END_DEPTH, MID_DEPTH - (k - 51))

            # S(0)'s second half, then the k0b half-projection
            with tc.high_priority():
                nc.tensor.matmul(
                    s0[:, 512:1024],
                    lhsT=kT_sb[0:DK, 0:128],
                    rhs=qT_sb[0:DK, 512:1024],
                    start=True,
                    stop=True,
                )
            s_next = s0
            proj_panel_half("k", 0, 1)
            for k in range(64):
                p, h, j = k // 32, (k // 16) % 2, k % 16
                if p not in ot_map:
                    ot_map[p] = otp_p.tile(
                        [128, IC * 128], dt.bfloat16, tag="otp", name=f"otp{p}"
                    )
                    y_pan[p] = xy_p.tile(
                        [128, KC * 1024], dt.bfloat16, tag="xy", name=f"ypan{p}"
                    )
                s_ps = s_next
                e0 = e_p.tile(
                    [128, 1024], dt.bfloat16, tag="e0", name=f"e0_{k}"
                )
                nc.scalar.activation(e0, s_ps, AF.Exp, scale=1.0 / math.sqrt(DK))
                eh = eh_p.tile(
                    [128, 1024], dt.bfloat16, tag="eh", name=f"eh{k}"
                )
                mslc = mask_slc(j, p, h)
                if fp8_k(k):
                    nc.vector.scalar_tensor_tensor(
                        eh, e0, 1.0, mslc, ALU.subtract, ALU.mult
                    )
                else:
                    t = t_p.tile(
                        [128, 1024], dt.bfloat16, tag="tm", name=f"tm{k}"
                    )
                    nc.vector.tensor_scalar_add(t, e0, -1.0)
                    if k in DVEMUL_K:
                        nc.vector.tensor_mul(eh, t, mslc)
                    else:
                        nc.gpsimd.tensor_mul(eh, t, mslc)
                # next S ahead of PV/side work so Act is never starved
                if k + 1 < 64:
                    s_next = emit_s(k + 1)
                # side work riding this iteration
                if k == 2:
                    proj_panel("k", 1)
                elif k == 6:
                    proj_panel_half("k", 2, 0)
                    proj_panel_half("k", 2, 1)
                elif k == 10:
                    proj_panel_half("k", 3, 0)
                    proj_panel_half("k", 3, 1)
                elif k == 28:
                    proj_panel_half("q", 2, 0)
                    proj_panel_half("q", 2, 1)
                elif k == 30:
                    proj_panel_half("q", 3, 0)
                    proj_panel_half("q", 3, 1)
                if 12 <= k <= 27:
                    v_proj_chunk(k - 12)
                # panel-0 y chunks ride the odd iterations of the second
                # half (oT_full panel 0 lands ~k=41); their DMAs go out in
                # row-pair groups as soon as both halves of a pair exist
                if P0_START <= k and k % 2 == 1:
                    ck = (k - P0_START) // 2
                    p0_chunk(ck, nc.vector)
                    if ck % 4 == 3:
                        y_dma(0, ck // 2 - 1, ck // 2 + 1)
                # variable-depth software pipeline for PV; after a head's
                # last j-block drains, pause 2 iterations so the norm ->
                # o-buffer-reuse chain overlaps the S stream instead of
                # stalling the next head's first PV
                if dstate["pause"] > 0:
                    dstate["pause"] -= 1
                else:
                    while len(pend) >= depth(k):
                        drain_one()
                        if dstate["pause"]:
                            break
                pend.append((p, h, j, eh))

            # remaining panel-0 y chunks: emitted before the PV flush so
            # their matmuls keep PE hot while the last PV/norm chain runs
            n_inb = max(0, (63 - P0_START) // 2 + 1)
            rows_dmad = 2 * sum(1 for c2 in range(n_inb) if c2 % 4 == 3)
            for ck in range(n_inb, 16):
                p0_chunk(ck, nc.vector if ck % 2 == 0 else nc.scalar)
                if ck % 2 == 1 and (ck + 1) // 2 - rows_dmad >= 2:
                    y_dma(0, rows_dmad, (ck + 1) // 2)
                    rows_dmad = (ck + 1) // 2
            if rows_dmad < 8:
                y_dma(0, rows_dmad, 8)

            while pend:
                drain_one()

            # ---- tail: panel-1 epilogue with PE transposes (PE and
            # all engines idle here; skips the 3us DMA-xbar latency) ----
            def y_dma_cols(p, half, lo, hi):
                nc.sync.dma_start(
                    out=yT_d[lo * 128 : hi * 128,
                             p * 1024 + half * 512 : p * 1024 + (half + 1) * 512]
                    .rearrange("(a p2) i -> p2 a i", p2=128),
                    in_=y_pan[p].rearrange("p (a i) -> p a i", i=1024)[
                        :, lo:hi, half * 512 : (half + 1) * 512
                    ],
                )

            o_ps = o_ps_map[TP - 1, HPC - 1]
            ot1 = ot_map[TP - 1]
            rr = (nc.vector, nc.scalar)

            def pe_transpose(lic):
                tp = ps_p.tile(
                    [128, 128], dt.bfloat16, tag="sps", name=f"tp{lic}"
                )
                nc.tensor.transpose(tp, ot1[:, lic * 128 : (lic + 1) * 128], ident)
                eng = nc.vector
                dst = oT_full[:, 1024 + lic * 128 : 1024 + (lic + 1) * 128]
                if eng is nc.scalar:
                    nc.scalar.activation(dst, tp, AF.Copy)
                else:
                    eng.tensor_copy(dst, tp)

            def tail_chunk(ck, half, nn=None):
                nn = ck % 8 if nn is None else nn
                eng = rr[ck % 2]
                pool, tag = (pv_p, "vps") if ck % 2 == 0 else (ps_p, "sps")
                if eng is nc.scalar:
                    y_ps = pool.tile(
                        [128, 512], dt.float32, tag=tag, name=f"y1_{ck}"
                    )
                    nc.tensor.matmul(
                        y_ps,
                        lhsT=wo_sb[:, nn * 128 : (nn + 1) * 128],
                        rhs=oT_full[:, 1024 + half * 512 : 1024 + (half + 1) * 512],
                        start=True,
                        stop=True,
                    )
                    nc.scalar.activation(
                        y_pan[1][:, nn * 1024 + half * 512 :
                                 nn * 1024 + (half + 1) * 512],
                        y_ps, AF.Copy,
                    )
                else:
                    o_chunk(1, nn, half, eng, pool=pool, tag=tag)

            norm_bank(TP - 1, HPC - 1, o_ps, ot1, 0, prio=True)
            for lic in range(7):
                pe_transpose(lic)
            for ck in range(8):
                tail_chunk(ck, 0)
                if ck == 3:
                    y_dma_cols(1, 0, 0, 4)
            y_dma_cols(1, 0, 4, 8)
            norm_bank(TP - 1, HPC - 1, o_ps, ot1, 1, prio=True)
            pe_transpose(7)
            for ck in range(8, 16):
                tail_chunk(ck, 1)
                if ck == 11:
                    y_dma_cols(1, 1, 0, 4)
                elif ck == 13:
                    y_dma_cols(1, 1, 4, 6)
                elif ck == 14:
                    y_dma_cols(1, 1, 6, 7)
            y_dma_cols(1, 1, 7, 8)

    nc.compile()
    return nc


def get_program():
    if "nc" not in _CACHE:
        _CACHE["nc"] = _build_program()
    return _CACHE["nc"]


def _wshuf(wT):
    """[1024 k, 128 n] -> [128 p, KC*128] with chunk kk at cols kk*128."""
    return np.ascontiguousarray(
        wT.reshape(KC, 128, DPC).transpose(1, 0, 2).reshape(128, KC * DPC)
    ).astype(BF16)


def make_in_maps(query, key, value, attention_mask, Wq, bq, Wk, bk, Wv, Wo):
    """Host-side sharding: per-core input dicts."""
    qT = np.ascontiguousarray(np.asarray(query, np.float32)[0].T).astype(BF16)
    kT = np.ascontiguousarray(np.asarray(key, np.float32)[0].T).astype(BF16)
    vT = np.ascontiguousarray(np.asarray(value, np.float32)[0].T).astype(BF16)
    maskTf = np.ascontiguousarray(np.asarray(attention_mask, np.float32)[0, 0].T)
    maskT8 = maskTf.astype(FP8)
    maskTb = maskTf.astype(BF16)
    # C = colsum(Vaug) per head = [colsum(value) @ Wv_h.T | S], fp64 on host,
    # split into bf16 hi+lo rows for near-fp32 injection accuracy
    vcol = np.asarray(value, np.float64)[0].sum(axis=0)  # [H]

    in_maps = []
    for c in range(NCORES):
        ns = slice(c * DPC, (c + 1) * DPC)
        cfull = vcol @ np.asarray(Wv, np.float64)[ns].T  # [DPC]
        cvec = np.zeros((2, HPC * VA), np.float64)
        for h in range(HPC):
            cvec[0, h * VA : h * VA + DK] = cfull[h * DK : (h + 1) * DK]
            cvec[0, h * VA + DK] = float(S)
        chi = cvec.astype(BF16)
        clo = (cvec - chi.astype(np.float64)).astype(BF16)
        cboth = np.concatenate([chi[0:1], clo[0:1]], axis=0)
        bqk = np.stack(
            [np.asarray(bq, np.float32)[ns], np.asarray(bk, np.float32)[ns]],
            axis=1,
        )
        in_maps.append(
            {
                "qT": qT,
                "kT": kT,
                "vT": vT,
                "maskT8": maskT8,
                "maskTb": maskTb,
                "wq": _wshuf(np.asarray(Wq, np.float32)[ns].T),
                "wk": _wshuf(np.asarray(Wk, np.float32)[ns].T),
                "wv": _wshuf(np.asarray(Wv, np.float32)[ns].T),
                "wo": np.ascontiguousarray(np.asarray(Wo, np.float32)[:, ns].T).astype(BF16),
                "bqk": np.ascontiguousarray(bqk),
                "cvec": cboth,
                "ident": np.eye(128, dtype=BF16),
            }
        )
    return in_maps


def combine_outputs(results, Wv_bias, Wo, bo):
    """Sum per-core partial yT's (bf16 -> fp32), add host-folded biases."""
    acc = np.zeros((H, S), np.float32)
    for r in results:
        acc += r["yT"].astype(np.float32)
    bias = np.asarray(bo, np.float32) + np.asarray(Wv_bias, np.float32) @ np.asarray(
        Wo, np.float32
    ).T
    return (acc.T + bias[None, :]).astype(np.float32)[None]


def kernel(
    query,
    key,
    value,
    attention_mask,
    Wq,
    bq,
    Wk,
    bk,
    Wv,
    bv,
    Wo,
    bo,
    head,
    hidden_size,
):
    from concourse.bass_utils import run_bass_kernel_spmd

    nc = get_program()
    in_maps = make_in_maps(
        query, key, value, attention_mask, Wq, bq, Wk, bk, Wv, Wo
    )
    res = run_bass_kernel_spmd(nc, in_maps, list(range(NCORES)))
    return combine_outputs(res.results, bv, Wo, bo)



# revision 126
# speedup vs baseline: 1.0132x; 1.0050x over previous
"""Multi-head attention (B=1, S=2048, H=1024, NH=16) on 8 trn2 NeuronCores.

Sharding: head-parallel. Core c owns heads {2c, 2c+1} (= 128 of the 1024
hidden dims). Each core computes its Q/K/V projection slices, the full
attention for its 2 heads, and a full-width partial of the output
projection (contraction over its 128 context dims). Host sums the 8
partials and adds the (host-folded) biases.

Attention elementwise path (the reference quirk: masked scores are set
to 0 pre-softmax, so masked lanes contribute exp(0)=1):

    E = m*exp(s/8) + (1-m)            (m in {0,1})
      = m*(e0 - 1) + 1,   e0 = exp(s/8)

  * Act engine: e0 = Exp(s_psum / 8) straight out of PSUM -> SBUF bf16,
    one 1024-col tile per iteration; Act is the body's rate limiter
    (64 x 1038 ns) and does ~nothing else until the epilogue.
  * Masked-combine, one of three per-tile paths (Pool has no PSUM
    access; fused stt runs 1x on DVE; plain tensor_scalar/tensor_tensor
    hit DVE 4x/2x modes with all-bf16 SBUF operands):
      - fp8 stt (1127ns DVE): fused (e0-1)*m, fp8 mask - most evens +
        panel-0 odd j<=9.
      - tsp+Pool: t = e0 - 1 on DVE at 4x (327ns), eh = t*m on Pool
        (2.2us, deep pipeline slack) - tiles placed in Pool's idle
        windows: ph0-even j0/2/4/8 (k=0..24) and all ph1 odds. ph0
        j12/j14 are dual-loaded (fp8 for h0, bf16 copy for h1) so h1's
        tiles ride Pool where DVE otherwise drifts behind Act.
      - tsp+DVE-mult (921ns): tiles feeding each head-handoff's norm
        chain (k=27-31) and the last tiles (61,63), where Pool's queue
        latency would stall the o-buffer handoff or the tail.
    Load-balance invariant: DVE+Act carry exp (66.4us) + combines +
    evictions ~ 146us between them; every avoidable DVE ns matters
    because Act can run only e_p=8 exps ahead (e0-buffer WAR), so mask
    lateness or DVE drift surfaces directly as Act stalls.
  * The "+1" term: sum_j 1*vaug[j,:] = colsum(Vaug) = C, an i-independent
    vector, injected into each PV PSUM accumulation as a single K=2
    matmul against host-precomputed C split into bf16 hi+lo rows.

Loop structure: one flat 64-iteration pipeline over (panel, head,
j-block) with panels of 1024 queries. S-matmuls are emitted one
iteration ahead (priority-0). PV pipeline depth: 14 while V streams
(k<30), dipping to 8 at k=36-41 to pull the p0h1->p1h0 norm handoff
ahead of its consumers, 9 mid, tapering to 3 at the end. V-projection
chunks ride k=13..28. Panel-0's y-chunks ride odd iterations k>=43
(oT panel 0 lands ~k=41 via xbar-transposes), their DMAs in row-pair
groups; 5 leftover chunks go just before the PV flush. Panel-1's y is
the tail: norm -> PE+identity transposes -> 16 chunks on DVE/Act with
column-half DMAs, last rows as 1-row DMAs. PSUM: s 2x2 + o 2 + y/v
2x1 = 8 banks.

DMA discipline (~360 GB/s serial transfer device, one HWDGE slot per
dma_start): the queue is deadline-sorted (EDF). Hard deadlines: x
loads gate proj->S->exp directly - prologue streams wk/wq + xk0/xq0/
xq1 as 256-token half-quarters chased by half-panel projections
(first exp ~13.5us), then xk1..3, then v quarters (PE-blocking via
the v-proj window). Soft deadlines: stt masks are due t(k + e_p) via
the e0-WAR; Pool-path masks only gate PV (t(k + depth)) and load in
the post-xq3 stream. Masks live in packed 1024-col slot tiles
(fp8 mask_e / bf16 mask_o + mo8/me16), y leaves in 0.25-0.5MB groups
as chunks complete. q/k biases fold into projection evictions.

Precision: identical to the reference-faithful baseline - all matmuls
bf16 with fp32 PSUM accumulation, softmax without max-subtraction
(exponent ~ N(0,0.33^2) cannot overflow). Modeled 110.5us (was 112.6);
hw rel err 1.4e-3.
"""

import math

import numpy as np
import ml_dtypes

BF16 = ml_dtypes.bfloat16
FP8 = ml_dtypes.float8_e4m3
S, H, NH, DK = 2048, 1024, 16, 64
NCORES = 8
HPC = NH // NCORES          # heads per core = 2
DPC = HPC * DK              # head dims per core = 128
KC = H // 128               # contraction chunks = 8
TP = 2                      # 1024-wide query token panels
JC = S // 128               # 128-wide key chunks = 16
IC = 1024 // 128            # i-chunks per panel = 8
VA = DK + 1                 # v columns + ones column = 65

_CACHE = {}

# schedule knobs (tuned via TimelineSim sweeps)
EARLY_DEPTH = 14     # PV pipeline depth while v-quarters stream
MID_DEPTH = 9        # depth after the early taper
END_DEPTH = 3        # post-loop drain backlog
P0_START = 43        # first in-body panel-0 y-chunk iteration (odd)
DVEMUL = (27, 29, 31, 61, 63)  # odd-j tiles multiplied on DVE
HANDOFF_PAUSE = 2    # iterations to pause draining after a head's last j
DIP_DEPTH = 8        # temporary depth dip at k=36-41 (handoff pull-in)
NORM_PRIO = 15     # priority offset for head-norm DVE ops (o-WAR release)


def _oslc(ic):
    """o_ps column offset for ic-th 65-wide slice: 7 slices in bank 0,
    the 8th at 512 so no matmul crosses a PSUM bank boundary."""
    b, r = divmod(ic, 7)
    return b * 512 + r * VA


def _build_program():
    """Build + compile the (identical) per-core Bass program."""
    from contextlib import ExitStack

    import concourse.bacc as bacc
    import concourse.tile as tile
    from concourse import mybir

    dt = mybir.dt
    AF = mybir.ActivationFunctionType
    ALU = mybir.AluOpType
    f8 = dt.float8e4

    nc = bacc.Bacc("TRN2", target_bir_lowering=False, debug=False)

    qT_d = nc.dram_tensor("qT", [H, S], dt.bfloat16, kind="ExternalInput").ap()
    kT_d = nc.dram_tensor("kT", [H, S], dt.bfloat16, kind="ExternalInput").ap()
    vT_d = nc.dram_tensor("vT", [H, S], dt.bfloat16, kind="ExternalInput").ap()
    maskT8_d = nc.dram_tensor("maskT8", [S, S], f8, kind="ExternalInput").ap()
    maskTb_d = nc.dram_tensor("maskTb", [S, S], dt.bfloat16, kind="ExternalInput").ap()
    wk_d = nc.dram_tensor("wk", [128, KC * DPC], dt.bfloat16, kind="ExternalInput").ap()
    wq_d = nc.dram_tensor("wq", [128, KC * DPC], dt.bfloat16, kind="ExternalInput").ap()
    wv_d = nc.dram_tensor("wv", [128, KC * DPC], dt.bfloat16, kind="ExternalInput").ap()
    wo_d = nc.dram_tensor("wo", [DPC, H], dt.bfloat16, kind="ExternalInput").ap()
    bqk_d = nc.dram_tensor("bqk", [128, 2], dt.float32, kind="ExternalInput").ap()
    cv_d = nc.dram_tensor("cvec", [2, HPC * VA], dt.bfloat16, kind="ExternalInput").ap()
    id_d = nc.dram_tensor("ident", [128, 128], dt.bfloat16, kind="ExternalInput").ap()
    yT_d = nc.dram_tensor("yT", [H, S], dt.bfloat16, kind="ExternalOutput").ap()

    with tile.TileContext(nc) as tc, ExitStack() as ctx:
        cp = ctx.enter_context(tc.tile_pool(name="const", bufs=1))
        e_p = ctx.enter_context(tc.tile_pool(name="ex", bufs=8))
        eh_p = ctx.enter_context(tc.tile_pool(name="ehat", bufs=15))
        rc_p = ctx.enter_context(tc.tile_pool(name="recip", bufs=2))
        t_p = ctx.enter_context(tc.tile_pool(name="tmul", bufs=3))
        mh_p = ctx.enter_context(tc.tile_pool(name="maskhi", bufs=1))
        otp_p = ctx.enter_context(tc.tile_pool(name="otpan", bufs=2))
        vin_p = ctx.enter_context(tc.tile_pool(name="vin", bufs=1))
        xy_p = ctx.enter_context(tc.tile_pool(name="xy", bufs=3))

        # ---- DMA schedule: wk | xk quarters | wq | xq quarters | rest ----
        wk_sb = cp.tile([128, KC * DPC], dt.bfloat16, tag="wk")
        nc.sync.dma_start(out=wk_sb, in_=wk_d)
        # preload the Exp activation table off the critical path
        warm = cp.tile([1, 2], dt.bfloat16, tag="warm")
        nc.vector.memset(warm, 0.0)
        nc.scalar.activation(warm, warm, AF.Exp)

        # ---- token-streamed inputs: x loads in token quarters so each
        # kT/qT panel completes as its quarter lands; S(j) needs only
        # kT token-block j and qT's active panel half, so attention
        # starts ~15us earlier. Late panels project as body side-work.
        wq_sb = cp.tile([128, KC * DPC], dt.bfloat16, tag="wq")
        bqk_sb = cp.tile([128, 2], dt.float32, tag="bqk")
        cv_sb = cp.tile([2, HPC * VA], dt.bfloat16, tag="cv")
        ident = cp.tile([128, 128], dt.bfloat16, tag="ident")
        qT_sb = cp.tile([128, S], dt.bfloat16, tag="qTs")
        kT_sb = cp.tile([128, S], dt.bfloat16, tag="kTs")
        vaug = cp.tile([128, JC * (HPC * VA)], dt.bfloat16, tag="vaug")
        nc.gpsimd.memset(
            vaug.rearrange("p (a v) -> p a v", v=VA)[:, :, DK:VA], 1.0
        )
        ot_pan = None
        oT_full = cp.tile([128, S], dt.bfloat16, tag="oTfull")
        y_pan = {}
        # packed mask slot layouts (1024-col slots), only the (j, ph)
        # combinations actually consumed from each dtype
        E_SLOT = {(6, 0): 0, (10, 0): 1, (12, 0): 2,
                  (14, 0): 3, (0, 1): 4, (2, 1): 5, (4, 1): 6, (6, 1): 7,
                  (8, 1): 8, (10, 1): 9, (12, 1): 10, (14, 1): 11}
        O_SLOT = {(11, 0): 0, (13, 0): 1, (15, 0): 2, (1, 1): 3, (3, 1): 4,
                  (5, 1): 5, (7, 1): 6, (9, 1): 7, (11, 1): 8, (13, 1): 9,
                  (15, 1): 10, (12, 0): 11, (14, 0): 12}
        mask_e = cp.tile([128, 12 * 1024], f8, tag="maske")
        mask_o = mh_p.tile([128, 13 * 1024], dt.bfloat16, tag="masko")
        wv_sb = cp.tile([128, KC * DPC], dt.bfloat16, tag="wv")
        wo_sb = cp.tile([128, H], dt.bfloat16, tag="wo")

        xtq = {}

        def x_tq(pre, x_d, c):
            xt = xy_p.tile(
                [128, KC * 1024], dt.bfloat16, tag="xy", name=f"x{pre}{c}"
            )[:, : KC * 512]
            nc.sync.dma_start(
                out=xt.rearrange("p (a i) -> p a i", a=KC),
                in_=x_d[:, c * 512 : (c + 1) * 512].rearrange(
                    "(a p) i -> p a i", p=128
                ),
            )
            xtq[pre, c] = xt

        def x_tq_half(pre, x_d, c, hf):
            """256-token half-quarter load (finer prologue pipelining)."""
            if (pre, c) not in xtq:
                xtq[pre, c] = xy_p.tile(
                    [128, KC * 1024], dt.bfloat16, tag="xy", name=f"x{pre}{c}"
                )[:, : KC * 512]
            nc.sync.dma_start(
                out=xtq[pre, c].rearrange("p (a i) -> p a i", a=KC)[
                    :, :, hf * 256 : (hf + 1) * 256
                ],
                in_=x_d[
                    :, c * 512 + hf * 256 : c * 512 + (hf + 1) * 256
                ].rearrange("(a p) i -> p a i", p=128),
            )

        def mask_cols(j0, nb, ph):
            """Load nb j-blocks of parity j0%2 starting at j0, cols half ph."""
            par = j0 % 2
            t, d, smap = (
                (mask_e, maskT8_d, E_SLOT) if par == 0
                else (mask_o, maskTb_d, O_SLOT)
            )
            slot = smap[j0, ph]
            nc.sync.dma_start(
                out=t.rearrange("p (b i) -> p b i", i=1024)[
                    :, slot : slot + nb, :
                ],
                in_=d.rearrange("(b two p) i -> p two b i", two=2, p=128)[
                    :, par, j0 // 2 : j0 // 2 + nb, ph * 1024 : (ph + 1) * 1024
                ],
            )

        # early odd-j (panel-0) mask slices staged fp8: they ride the fused
        # stt path, saving deadline-critical early DMA
        mo8 = cp.tile([128, 5 * 1024], f8, tag="mo8")
        # ph0 evens j0, j2, j4, j8 staged bf16: their tiles (k=0,2,4,8 and
        # 16,18,20,24) ride the tsp+Pool path in Pool's idle windows,
        # relieving DVE's 1127ns stt load where it drifts behind Act
        ME_SLOT = {0: 0, 2: 1, 4: 2, 8: 3}
        me16 = cp.tile([128, 4 * 1024], dt.bfloat16, tag="me16")

        def mask_slc(j, ph, h=0):
            if ph == 0 and j % 2 == 1 and j <= 9:
                return mo8[:, (j // 2) * 1024 : (j // 2 + 1) * 1024]
            if ph == 0 and j in ME_SLOT:
                s_ = ME_SLOT[j]
                return me16[:, s_ * 1024 : (s_ + 1) * 1024]
            if ph == 0 and h == 1 and j in (12, 14):
                # h1's copy of these slices is bf16 (Pool path); h0 uses fp8
                slot = O_SLOT[j, 0]
                return mask_o[:, slot * 1024 : (slot + 1) * 1024]
            t, smap = (mask_e, E_SLOT) if j % 2 == 0 else (mask_o, O_SLOT)
            slot = smap[j, ph]
            return t[:, slot * 1024 : (slot + 1) * 1024]

        vin = []

        def v_quarter(c):
            t_ = vin_p.tile(
                [128, KC * 512], dt.bfloat16, tag=f"vq{c % 3}", name=f"vq{c}"
            )
            nc.sync.dma_start(
                out=t_.rearrange("p (a i) -> p a i", a=KC),
                in_=vT_d[:, c * 512 : (c + 1) * 512].rearrange(
                    "(a p) i -> p a i", p=128
                ),
            )
            vin.append(t_)

        # DMA queue order: deadline-sorted (EDF) just-in-time stream over
        # the serial ~360GB/s transfer device. x-loads gate the S->exp
        # chain directly (hard deadlines); masks/v have pipeline slack.
        x_tq_half("k", kT_d, 0, 0)
        nc.sync.dma_start(out=wq_sb, in_=wq_d)
        nc.sync.dma_start(out=bqk_sb, in_=bqk_d)
        x_tq_half("q", qT_d, 0, 0)
        x_tq_half("q", qT_d, 0, 1)
        x_tq_half("q", qT_d, 1, 0)
        x_tq_half("q", qT_d, 1, 1)
        x_tq_half("k", kT_d, 0, 1)
        x_tq("k", kT_d, 1)
        x_tq_half("k", kT_d, 2, 0)
        x_tq_half("k", kT_d, 2, 1)
        nc.sync.dma_start(              # j0, j2 ph0 bf16 (Pool path)
            out=me16[:, 0 : 2 * 1024].rearrange("p (b i) -> p b i", i=1024),
            in_=maskTb_d.rearrange("(b two p) i -> p two b i", two=2, p=128)[
                :, 0, 0:2, 0:1024
            ],
        )
        nc.sync.dma_start(              # j1,3,5,7,9 ph0 as fp8
            out=mo8.rearrange("p (b i) -> p b i", i=1024),
            in_=maskT8_d.rearrange("(b two p) i -> p two b i", two=2, p=128)[
                :, 1, 0:5, 0:1024
            ],
        )
        x_tq_half("k", kT_d, 3, 0)
        x_tq_half("k", kT_d, 3, 1)
        nc.sync.dma_start(out=wv_sb, in_=wv_d)
        nc.sync.dma_start(              # j4 ph0 bf16 (Pool path)
            out=me16[:, 2 * 1024 : 3 * 1024],
            in_=maskTb_d.rearrange("(b two p) i -> p two b i", two=2, p=128)[
                :, 0, 2, 0:1024
            ],
        )
        mask_cols(6, 1, 0)              # j6 (fp8)
        v_quarter(0)
        nc.sync.dma_start(              # j8 ph0 bf16 (Pool path)
            out=me16[:, 3 * 1024 : 4 * 1024],
            in_=maskTb_d.rearrange("(b two p) i -> p two b i", two=2, p=128)[
                :, 0, 4, 0:1024
            ],
        )
        mask_cols(10, 1, 0)             # j10 (fp8)
        v_quarter(1)
        mask_cols(11, 2, 0)             # j11, j13 (bf16)
        mask_cols(12, 2, 0)             # j12, j14 (fp8)
        v_quarter(2)
        mask_cols(15, 1, 0)             # j15 (bf16)
        v_quarter(3)
        x_tq_half("q", qT_d, 2, 0)
        x_tq_half("q", qT_d, 2, 1)
        x_tq_half("q", qT_d, 3, 0)
        x_tq_half("q", qT_d, 3, 1)
        nc.sync.dma_start(              # j12, j14 ph0 bf16 copies (h1 Pool)
            out=mask_o.rearrange("p (b i) -> p b i", i=1024)[:, 11:13, :],
            in_=maskTb_d.rearrange("(b two p) i -> p two b i", two=2, p=128)[
                :, 0, 6:8, 0:1024
            ],
        )
        nc.sync.dma_start(out=cv_sb, in_=cv_d)
        mask_cols(0, 2, 1)              # ph1 evens j0, j2
        mask_cols(1, 2, 1)              # ph1 odds j1, j3
        mask_cols(4, 2, 1)
        mask_cols(5, 2, 1)
        nc.sync.dma_start(out=wo_sb, in_=wo_d)
        mask_cols(8, 2, 1)
        mask_cols(9, 2, 1)
        mask_cols(12, 2, 1)
        mask_cols(13, 2, 1)
        nc.sync.dma_start(out=ident, in_=id_d)

        ones2 = cp.tile([2, 128], dt.bfloat16, tag="ones2")
        nc.vector.memset(ones2, 1.0)

        # ---- attention: PSUM = s 2x2 + o 1x2 + y/v 2x1 = 8 banks ----
        with tc.tile_pool(name="ps_s", bufs=2, space="PSUM") as ps_p, \
             tc.tile_pool(name="ps_o", bufs=1, space="PSUM") as po_p, \
             tc.tile_pool(name="ps_v", bufs=2, space="PSUM") as pv_p:

            def v_proj_chunk(t):
                """Token-chunk t of the V projection into vaug."""
                ps = pv_p.tile([128, DPC], dt.float32, tag="vps", name=f"pv{t}")
                c, ts_ = divmod(t, 4)
                for kk in range(KC):
                    nc.tensor.matmul(
                        ps,
                        lhsT=vin[c][:, kk * 512 + ts_ * 128 : kk * 512 + (ts_ + 1) * 128],
                        rhs=wv_sb[:, kk * DPC : (kk + 1) * DPC],
                        start=(kk == 0),
                        stop=(kk == KC - 1),
                    )
                base = t * (HPC * VA)
                dst = vaug[:, base : base + HPC * VA].rearrange(
                    "p (a v) -> p a v", v=VA
                )[:, :, 0:DK]
                src = ps.rearrange("p (a d) -> p a d", d=DK)
                if t >= 12:
                    # late chunks evict on Act: DVE is the pacer by then
                    # and Act idles waiting on the panel-1 q stream
                    nc.scalar.activation(dst, src, AF.Copy)
                else:
                    nc.vector.tensor_copy(dst, src)

            def pv_mms(h, j, et, o_ps):
                for ic in range(IC):
                    nc.tensor.matmul(
                        o_ps[:, _oslc(ic) : _oslc(ic) + VA],
                        lhsT=et[:, ic * 128 : (ic + 1) * 128],
                        rhs=vaug[:, j * (HPC * VA) + h * VA : j * (HPC * VA) + (h + 1) * VA],
                        start=(j == 0 and ic % 7 == 0),
                        stop=False,
                    )

            def c_inject(h, o_ps):
                """+C (hi+lo rows); last slice per bank carries the stop."""
                for ic in range(IC):
                    nc.tensor.matmul(
                        o_ps[:, _oslc(ic) : _oslc(ic) + VA],
                        lhsT=ones2,
                        rhs=cv_sb[:, h * VA : (h + 1) * VA],
                        start=False,
                        stop=(ic in (6, 7)),
                    )

            import concourse.bass as bass_mod

            def norm_bank(p, h, o_ps, ot_pan, b, prio=False):
                """Normalize one PSUM bank of o_ps into ot_pan. With prio,
                schedule the DVE ops early in the stream: the norm releases
                the o-PSUM buffer the next head's PV (and thus the whole PE
                stream) waits on."""
                ctx2 = tc.high_priority(offset=NORM_PRIO) if prio else None
                if ctx2 is not None:
                    ctx2.__enter__()
                try:
                    n_ic = (7, 1)[b]
                    rc = rc_p.tile(
                        [128, 8], dt.float32, tag="rc", name=f"rc{p}_{h}_{b}"
                    )
                    den = bass_mod.AP(
                        tensor=o_ps.tensor,
                        offset=o_ps.offset + b * 512 + DK,
                        ap=[o_ps.ap[0], [VA, n_ic]],
                    )
                    nc.vector.reciprocal(rc[:, :n_ic], den)
                    src_ap = bass_mod.AP(
                        tensor=o_ps.tensor,
                        offset=o_ps.offset + b * 512,
                        ap=[o_ps.ap[0], [VA, n_ic], [1, DK]],
                    )
                    rcb = bass_mod.AP(
                        tensor=rc.tensor,
                        offset=rc.offset,
                        ap=[rc.ap[0], [1, n_ic], [0, DK]],
                    )
                    dst = bass_mod.AP(
                        tensor=ot_pan.tensor,
                        offset=ot_pan.offset + b * 7 * 128 + h * DK,
                        ap=[ot_pan.ap[0], [128, n_ic], [1, DK]],
                    )
                    nc.vector.tensor_mul(dst, src_ap, rcb)
                finally:
                    if ctx2 is not None:
                        ctx2.__exit__(None, None, None)

            def o_chunk(p, nn, half, eng, pool=None, tag="vps"):
                """One 512-col y chunk: matmul + eviction into y_pan."""
                y_ps = (pool or pv_p).tile(
                    [128, 512], dt.float32, tag=tag, name=f"y{p}_{nn}_{half}"
                )
                nc.tensor.matmul(
                    y_ps,
                    lhsT=wo_sb[:, nn * 128 : (nn + 1) * 128],
                    rhs=oT_full[:, p * 1024 + half * 512 : p * 1024 + (half + 1) * 512],
                    start=True,
                    stop=True,
                )
                eng.tensor_copy(
                    y_pan[p][:, nn * 1024 + half * 512 : nn * 1024 + (half + 1) * 512],
                    y_ps,
                )

            def y_dma(p, lo, hi):
                """DMA y_pan[p] rows nn in [lo,hi) out to yT."""
                nc.sync.dma_start(
                    out=yT_d[lo * 128 : hi * 128, p * 1024 : (p + 1) * 1024]
                    .rearrange("(a p2) i -> p2 a i", p2=128),
                    in_=y_pan[p][:, lo * 1024 : hi * 1024]
                    .rearrange("p (a i) -> p a i", i=1024),
                )

            # ---- flat 64-iteration pipeline ----
            ot_map = {}

            def emit_s(k):
                p, h, j = k // 32, (k // 16) % 2, k % 16
                hs = h * DK
                s_ps = ps_p.tile(
                    [128, 1024], dt.float32, tag="sps", name=f"s{k}"
                )
                with tc.high_priority():
                    for q in range(2):
                        nc.tensor.matmul(
                            s_ps[:, q * 512 : (q + 1) * 512],
                            lhsT=kT_sb[hs : hs + DK, j * 128 : (j + 1) * 128],
                            rhs=qT_sb[hs : hs + DK,
                                      p * 1024 + q * 512 : p * 1024 + (q + 1) * 512],
                            start=True,
                            stop=True,
                        )
                return s_ps

            def transposes(p):
                # batched xbar transposes, bank-0's 7 chunks first
                nc.sync.dma_start_transpose(
                    out=oT_full[:, p * 1024 : p * 1024 + 896].rearrange(
                        "p2 (b c) -> p2 b c", c=128
                    ),
                    in_=ot_map[p][:, 0:896],
                )
                nc.sync.dma_start_transpose(
                    out=oT_full[:, p * 1024 + 896 : (p + 1) * 1024],
                    in_=ot_map[p][:, 896:1024],
                )

            def norm_banks(p, h, o_ps, ot_pan):
                norm_bank(p, h, o_ps, ot_pan, 0)
                norm_bank(p, h, o_ps, ot_pan, 1)

            o_ps_map = {}
            pend = []
            dstate = {"pause": 0}

            def drain_one():
                pp, ph, pj, peh = pend.pop(0)
                if (pp, ph) not in o_ps_map:
                    o_ps_map[pp, ph] = po_p.tile(
                        [128, 1024], dt.float32, tag="ops", name=f"ops{pp}{ph}"
                    )
                o_ps = o_ps_map[pp, ph]
                pv_mms(ph, pj, peh, o_ps)
                if pj == JC - 1:
                    c_inject(ph, o_ps)
                    if (pp, ph) != (TP - 1, HPC - 1):
                        norm_banks(pp, ph, o_ps, ot_map[pp])
                        if ph == HPC - 1:
                            transposes(pp)
                        dstate["pause"] = HANDOFF_PAUSE

            def proj_panel(pre, c):
                w, dest = (wk_sb, kT_sb) if pre == "k" else (wq_sb, qT_sb)
                bcol = bqk_sb[:, 1:2] if pre == "k" else bqk_sb[:, 0:1]
                ps = pv_p.tile(
                    [128, 512], dt.float32, tag="vps", name=f"pp{pre}{c}"
                )
                for kk in range(KC):
                    nc.tensor.matmul(
                        ps,
                        lhsT=w[:, kk * DPC : (kk + 1) * DPC],
                        rhs=xtq[pre, c][:, kk * 512 : (kk + 1) * 512],
                        start=(kk == 0),
                        stop=(kk == KC - 1),
                    )
                # DVE eviction: an Act Identity here would displace an exp
                nc.vector.tensor_scalar_add(
                    dest[:, c * 512 : (c + 1) * 512], ps, bcol
                )

            def proj_panel_half(pre, c, hf, act=False):
                """256-token half-panel projection (prologue pipelining)."""
                w, dest = (wk_sb, kT_sb) if pre == "k" else (wq_sb, qT_sb)
                bcol = bqk_sb[:, 1:2] if pre == "k" else bqk_sb[:, 0:1]
                ps = pv_p.tile(
                    [128, 256], dt.float32, tag="vps", name=f"ph{pre}{c}{hf}"
                )
                lo = hf * 256
                for kk in range(KC):
                    nc.tensor.matmul(
                        ps,
                        lhsT=w[:, kk * DPC : (kk + 1) * DPC],
                        rhs=xtq[pre, c][:, kk * 512 + lo : kk * 512 + lo + 256],
                        start=(kk == 0),
                        stop=(kk == KC - 1),
                    )
                if act:
                    nc.scalar.activation(
                        dest[:, c * 512 + lo : c * 512 + lo + 256], ps,
                        AF.Identity, bias=bcol,
                    )
                else:
                    nc.vector.tensor_scalar_add(
                        dest[:, c * 512 + lo : c * 512 + lo + 256], ps, bcol
                    )

            proj_panel_half("k", 0, 0)
            proj_panel_half("q", 0, 0)
            proj_panel_half("q", 0, 1)
            # S(0)'s first half only needs qT cols 0-511: run it while the
            # xq1 halves are still streaming in
            s0 = ps_p.tile([128, 1024], dt.float32, tag="sps", name="s0")
            with tc.high_priority():
                nc.tensor.matmul(
                    s0[:, 0:512],
                    lhsT=kT_sb[0:DK, 0:128],
                    rhs=qT_sb[0:DK, 0:512],
                    start=True,
                    stop=True,
                )
            proj_panel_half("q", 1, 0)
            proj_panel_half("q", 1, 1)

            def p0_chunk(ck, eng):
                """One 512-col panel-0 y chunk; eviction on `eng`."""
                nn, half = ck // 2, ck % 2
                y_ps = pv_p.tile(
                    [128, 512], dt.float32, tag="vps", name=f"y0_{ck}"
                )
                nc.tensor.matmul(
                    y_ps,
                    lhsT=wo_sb[:, nn * 128 : (nn + 1) * 128],
                    rhs=oT_full[:, half * 512 : (half + 1) * 512],
                    start=True,
                    stop=True,
                )
                if eng is nc.scalar:
                    nc.scalar.activation(
                        y_pan[0][:, nn * 1024 + half * 512 :
                                 nn * 1024 + (half + 1) * 512],
                        y_ps, AF.Copy,
                    )
                else:
                    eng.tensor_copy(
                        y_pan[0][:, nn * 1024 + half * 512 :
                                 nn * 1024 + (half + 1) * 512],
                        y_ps,
                    )

            # per-iteration elementwise path:
            #  - fp8 stt (fused, 1x DVE): all even-j + panel-0 odd j<=9
            #  - tsp(e0-1) 4x + DVE 2x mult: tiles feeding the panel-0 norm
            #    chain (pull eh27-31 early) and the last tiles (Pool lags)
            #  - tsp(e0-1) 4x + Pool mult: everything else
            # DVE-mult tiles: the last odd-j of each head feed the norm ->
            # next-head-PV chain (o-PSUM buffer reuse); Pool's lag there
            # would stall the S stream at every head handoff
            DVEMUL_K = set(DVEMUL)

            def fp8_k(k):
                j = k % 16
                if k < 32 and j in (0, 2, 4, 8):
                    return False  # bf16, tsp+Pool in Pool's idle windows
                if 16 <= k < 32 and j in (12, 14):
                    return False  # dual-loaded: h1 copy is bf16 Pool path
                return j % 2 == 0 or (k < 32 and j <= 9)

            def depth(k):
                """PV pipeline depth: deep early (v-load slack), shallower
                mid (pulls the panel-0 norm chain ahead of its y consumers),
                tapering at the end so the post-loop backlog stays short."""
                if k < 30:
                    return EARLY_DEPTH
                if k < 52:
                    d = max(MID_DEPTH, EARLY_DEPTH - 2 * (k - 29))
                    if 36 <= k < 42:
                        d = min(d, DIP_DEPTH)
                    return d
                return max(END_DEPTH, MID_DEPTH - (k - 51))

            # S(0)'s second half, then the k0b half-projection
            with tc.high_priority():
                nc.tensor.matmul(
                    s0[:, 512:1024],
                    lhsT=kT_sb[0:DK, 0:128],
                    rhs=qT_sb[0:DK, 512:1024],
                    start=True,
                    stop=True,
                )
            s_next = s0
            proj_panel_half("k", 0, 1)
            for k in range(64):
                p, h, j = k // 32, (k // 16) % 2, k % 16
                if p not in ot_map:
                    ot_map[p] = otp_p.tile(
                        [128, IC * 128], dt.bfloat16, tag="otp", name=f"otp{p}"
                    )
                    y_pan[p] = xy_p.tile(
                        [128, KC * 1024], dt.bfloat16, tag="xy", name=f"ypan{p}"
                    )
                s_ps = s_next
                e0 = e_p.tile(
                    [128, 1024], dt.bfloat16, tag="e0", name=f"e0_{k}"
                )
                nc.scalar.activation(e0, s_ps, AF.Exp, scale=1.0 / math.sqrt(DK))
                eh = eh_p.tile(
                    [128, 1024], dt.bfloat16, tag="eh", name=f"eh{k}"
                )
                mslc = mask_slc(j, p, h)
                if fp8_k(k):
                    nc.vector.scalar_tensor_tensor(
                        eh, e0, 1.0, mslc, ALU.subtract, ALU.mult
                    )
                else:
                    t = t_p.tile(
                        [128, 1024], dt.bfloat16, tag="tm", name=f"tm{k}"
                    )
                    nc.vector.tensor_scalar_add(t, e0, -1.0)
                    if k in DVEMUL_K:
                        nc.vector.tensor_mul(eh, t, mslc)
                    else:
                        nc.gpsimd.tensor_mul(eh, t, mslc)
                # next S ahead of PV/side work so Act is never starved
                if k + 1 < 64:
                    s_next = emit_s(k + 1)
                # side work riding this iteration
                if k == 2:
                    proj_panel("k", 1)
                elif k == 6:
                    proj_panel_half("k", 2, 0)
                    proj_panel_half("k", 2, 1)
                elif k == 10:
                    proj_panel_half("k", 3, 0)
                    proj_panel_half("k", 3, 1)
                elif k == 28:
                    proj_panel_half("q", 2, 0)
                    proj_panel_half("q", 2, 1)
                elif k == 30:
                    proj_panel_half("q", 3, 0)
                    proj_panel_half("q", 3, 1)
                if 11 <= k <= 26:
                    v_proj_chunk(k - 11)
                # panel-0 y chunks ride the odd iterations of the second
                # half (oT_full panel 0 lands ~k=41); their DMAs go out in
                # row-pair groups as soon as both halves of a pair exist
                if P0_START <= k and k % 2 == 1:
                    ck = (k - P0_START) // 2
                    p0_chunk(ck, nc.vector)
                    if ck % 4 == 3:
                        y_dma(0, ck // 2 - 1, ck // 2 + 1)
                # variable-depth software pipeline for PV; after a head's
                # last j-block drains, pause 2 iterations so the norm ->
                # o-buffer-reuse chain overlaps the S stream instead of
                # stalling the next head's first PV
                if dstate["pause"] > 0:
                    dstate["pause"] -= 1
                else:
                    while len(pend) >= depth(k):
                        drain_one()
                        if dstate["pause"]:
                            break
                pend.append((p, h, j, eh))

            # remaining panel-0 y chunks: emitted before the PV flush so
            # their matmuls keep PE hot while the last PV/norm chain runs
            n_inb = max(0, (63 - P0_START) // 2 + 1)
            rows_dmad = 2 * sum(1 for c2 in range(n_inb) if c2 % 4 == 3)
            for ck in range(n_inb, 16):
                p0_chunk(ck, nc.vector if ck % 2 == 0 else nc.scalar)
                if ck % 2 == 1 and (ck + 1) // 2 - rows_dmad >= 2:
                    y_dma(0, rows_dmad, (ck + 1) // 2)
                    rows_dmad = (ck + 1) // 2
            if rows_dmad < 8:
                y_dma(0, rows_dmad, 8)

            while pend:
                drain_one()

            # ---- tail: panel-1 epilogue with PE transposes (PE and
            # all engines idle here; skips the 3us DMA-xbar latency) ----
            def y_dma_cols(p, half, lo, hi):
                nc.sync.dma_start(
                    out=yT_d[lo * 128 : hi * 128,
                             p * 1024 + half * 512 : p * 1024 + (half + 1) * 512]
                    .rearrange("(a p2) i -> p2 a i", p2=128),
                    in_=y_pan[p].rearrange("p (a i) -> p a i", i=1024)[
                        :, lo:hi, half * 512 : (half + 1) * 512
                    ],
                )

            o_ps = o_ps_map[TP - 1, HPC - 1]
            ot1 = ot_map[TP - 1]
            rr = (nc.vector, nc.scalar)

            def pe_transpose(lic):
                tp = ps_p.tile(
                    [128, 128], dt.bfloat16, tag="sps", name=f"tp{lic}"
                )
                nc.tensor.transpose(tp, ot1[:, lic * 128 : (lic + 1) * 128], ident)
                eng = nc.vector
                dst = oT_full[:, 1024 + lic * 128 : 1024 + (lic + 1) * 128]
                if eng is nc.scalar:
                    nc.scalar.activation(dst, tp, AF.Copy)
                else:
                    eng.tensor_copy(dst, tp)

            def tail_chunk(ck, half, nn=None):
                nn = ck % 8 if nn is None else nn
                eng = rr[ck % 2]
                pool, tag = (pv_p, "vps") if ck % 2 == 0 else (ps_p, "sps")
                if eng is nc.scalar:
                    y_ps = pool.tile(
                        [128, 512], dt.float32, tag=tag, name=f"y1_{ck}"
                    )
                    nc.tensor.matmul(
                        y_ps,
                        lhsT=wo_sb[:, nn * 128 : (nn + 1) * 128],
                        rhs=oT_full[:, 1024 + half * 512 : 1024 + (half + 1) * 512],
                        start=True,
                        stop=True,
                    )
                    nc.scalar.activation(
                        y_pan[1][:, nn * 1024 + half * 512 :
                                 nn * 1024 + (half + 1) * 512],
                        y_ps, AF.Copy,
                    )
                else:
                    o_chunk(1, nn, half, eng, pool=pool, tag=tag)

            norm_bank(TP - 1, HPC - 1, o_ps, ot1, 0, prio=True)
            for lic in range(7):
                pe_transpose(lic)
            for ck in range(8):
                tail_chunk(ck, 0)
                if ck == 3:
                    y_dma_cols(1, 0, 0, 4)
            y_dma_cols(1, 0, 4, 8)
            norm_bank(TP - 1, HPC - 1, o_ps, ot1, 1, prio=True)
            pe_transpose(7)
            for ck in range(8, 16):
                tail_chunk(ck, 1)
                if ck == 11:
                    y_dma_cols(1, 1, 0, 4)
                elif ck == 13:
                    y_dma_cols(1, 1, 4, 6)
                elif ck == 14:
                    y_dma_cols(1, 1, 6, 7)
            y_dma_cols(1, 1, 7, 8)

    nc.compile()
    return nc


def get_program():
    if "nc" not in _CACHE:
        _CACHE["nc"] = _build_program()
    return _CACHE["nc"]


def _wshuf(wT):
    """[1024 k, 128 n] -> [128 p, KC*128] with chunk kk at cols kk*128."""
    return np.ascontiguousarray(
        wT.reshape(KC, 128, DPC).transpose(1, 0, 2).reshape(128, KC * DPC)
    ).astype(BF16)


def make_in_maps(query, key, value, attention_mask, Wq, bq, Wk, bk, Wv, Wo):
    """Host-side sharding: per-core input dicts."""
    qT = np.ascontiguousarray(np.asarray(query, np.float32)[0].T).astype(BF16)
    kT = np.ascontiguousarray(np.asarray(key, np.float32)[0].T).astype(BF16)
    vT = np.ascontiguousarray(np.asarray(value, np.float32)[0].T).astype(BF16)
    maskTf = np.ascontiguousarray(np.asarray(attention_mask, np.float32)[0, 0].T)
    maskT8 = maskTf.astype(FP8)
    maskTb = maskTf.astype(BF16)
    # C = colsum(Vaug) per head = [colsum(value) @ Wv_h.T | S], fp64 on host,
    # split into bf16 hi+lo rows for near-fp32 injection accuracy
    vcol = np.asarray(value, np.float64)[0].sum(axis=0)  # [H]

    in_maps = []
    for c in range(NCORES):
        ns = slice(c * DPC, (c + 1) * DPC)
        cfull = vcol @ np.asarray(Wv, np.float64)[ns].T  # [DPC]
        cvec = np.zeros((2, HPC * VA), np.float64)
        for h in range(HPC):
            cvec[0, h * VA : h * VA + DK] = cfull[h * DK : (h + 1) * DK]
            cvec[0, h * VA + DK] = float(S)
        chi = cvec.astype(BF16)
        clo = (cvec - chi.astype(np.float64)).astype(BF16)
        cboth = np.concatenate([chi[0:1], clo[0:1]], axis=0)
        bqk = np.stack(
            [np.asarray(bq, np.float32)[ns], np.asarray(bk, np.float32)[ns]],
            axis=1,
        )
        in_maps.append(
            {
                "qT": qT,
                "kT": kT,
                "vT": vT,
                "maskT8": maskT8,
                "maskTb": maskTb,
                "wq": _wshuf(np.asarray(Wq, np.float32)[ns].T),
                "wk": _wshuf(np.asarray(Wk, np.float32)[ns].T),
                "wv": _wshuf(np.asarray(Wv, np.float32)[ns].T),
                "wo": np.ascontiguousarray(np.asarray(Wo, np.float32)[:, ns].T).astype(BF16),
                "bqk": np.ascontiguousarray(bqk),
                "cvec": cboth,
                "ident": np.eye(128, dtype=BF16),
            }
        )
    return in_maps


def combine_outputs(results, Wv_bias, Wo, bo):
    """Sum per-core partial yT's (bf16 -> fp32), add host-folded biases."""
    acc = np.zeros((H, S), np.float32)
    for r in results:
        acc += r["yT"].astype(np.float32)
    bias = np.asarray(bo, np.float32) + np.asarray(Wv_bias, np.float32) @ np.asarray(
        Wo, np.float32
    ).T
    return (acc.T + bias[None, :]).astype(np.float32)[None]


def kernel(
    query,
    key,
    value,
    attention_mask,
    Wq,
    bq,
    Wk,
    bk,
    Wv,
    bv,
    Wo,
    bo,
    head,
    hidden_size,
):
    from concourse.bass_utils import run_bass_kernel_spmd

    nc = get_program()
    in_maps = make_in_maps(
        query, key, value, attention_mask, Wq, bq, Wk, bk, Wv, Wo
    )
    res = run_bass_kernel_spmd(nc, in_maps, list(range(NCORES)))
    return combine_outputs(res.results, bv, Wo, bo)

